# revision 10
# baseline (speedup 1.0000x reference)
"""Trainium2 Bass kernel for nn_MentionScore.

Strategy: sort spans by start, shard 2048 consecutive sorted spans per core.
Each core only touches a ~1.1k-token window of states/embeds (host passes the
window pre-transposed, bf16). The ragged gather/softmax/weighted-sum becomes
dense matmuls against one-hot / banded matrices built on-device with
iota-compare tensor ops. Layer-1 of the span MLP is algebraically folded:
  h1 = relu(OH_s.T@P1 + OH_e.T@P2 + W.T@P3' + onehot(len).T@WB)
with P1=states@W1a, P2=states@W1b, P3'=exp(attn)*(embeds@W1c) per token and
WB = width_table@W1d + b1.

Softmax separability: exp(span_attns[n,l]) = exp(attns[start_n+l]) means the
softmax numerator matrix factors as diag(exp(attns)) @ Band where Band is the
0/1 in-range indicator. exp() is folded into P3 per token (P3'), the
denominator ssum = Band.T @ exp(attns) comes from an N=1 matmul riding the
Band stationary, and the weight band is W = Band * broadcast(1/ssum), with the
broadcast done by a K=1 PE outer product.
"""

import sys
import types

import numpy as np
import ml_dtypes

import concourse.bass as bass
import concourse.mybir as mybir
from concourse.ap import AP
from concourse.tile import TileContext
from concourse.vector_clock import ScopedClock

BF = mybir.dt.bfloat16
F16 = mybir.dt.float16
F32 = mybir.dt.float32
AT = mybir.AluOpType
AF = mybir.ActivationFunctionType
AX = mybir.AxisListType
bf16 = ml_dtypes.bfloat16

N_CORES = 8
T, NSPAN, D, HID, LMAX, WD = 8192, 16384, 1024, 1024, 10, 20
C = NSPAN // N_CORES          # spans per core
G = C // 128                  # 128-span groups per core


class PatchedTileContext(TileContext):
    """Workaround: walrus rejects the tail Drain when it carries >1 sem wait
    ("Too many sync wait commands"). Put each wait on its own NoOp instead."""

    def _drain_and_barrier(self, tick_clock, wait_clock):
        nc = self.nc
        drain_inst = nc.sync.drain()
        wait_clock.add_sem_waits(
            drain_inst.ins, ScopedClock({None: tick_clock.global_clock})
        )
        si = drain_inst.ins.sync_info
        if si is not None and si.on_wait is not None and len(si.on_wait) > 1:
            waits = list(si.on_wait)
            drain_inst.ins.sync_info = mybir.SyncInfo(
                on_wait=[waits[0]], on_update=list(si.on_update or [])
            )
            for w in waits[1:]:
                nop = nc.sync.nop()
                nop.ins.sync_info = mybir.SyncInfo(on_wait=[w], on_update=[])

        nc.all_engine_barrier()
        assert self.sems is not None
        popped = nc._tile_sem_poison_stack.pop()
        assert popped is self._sem_poison
        nc.clear_and_free_semaphores(list(self.sems.allocated().values()))
        nc.all_engine_barrier()


def _ceil128(x):
    return int(-(-int(x) // 128) * 128)


def _plan(span_starts, span_lengths):
    """Host-side sharding plan. Returns per-core data + static layout consts."""
    order = np.argsort(span_starts, kind="stable").astype(np.int64)
    ss = span_starts[order].reshape(N_CORES, C).astype(np.int64)
    sl = span_lengths[order].reshape(N_CORES, C).astype(np.int64)
    core_base = ss[:, 0].copy()
    sloc = ss - core_base[:, None]
    eloc = sloc + sl

    T_cap = _ceil128(int(eloc.max()) + 1)
    # unaligned, shared-across-cores group window bases + per-group k-tiles
    mn = sloc[:, ::128].min(axis=0)                       # [G]
    mx = eloc.reshape(N_CORES, G, 128).max(axis=2).max(axis=0)  # [G]
    need = mx - mn + 1
    kcs = np.maximum((need + 127) // 128, 1)
    T_pad = T_cap + 128
    bases = mn.copy()
    for _ in range(3):
        bases = np.minimum(mn, T_pad - kcs * 128)
        bad = (mx - bases + 1) > kcs * 128
        if not bad.any():
            break
        kcs[bad] += 1
    K_WIN = int(kcs.max()) * 128
    d = sloc - np.repeat(bases, 128)[None, :]
    assert d.min() >= 0 and ((d + sl).reshape(N_CORES, G, 128).max(axis=2)
                             <= kcs[None, :] * 128 - 1).all(), "window overflow"

    return {
        "order": order,
        "core_base": core_base,
        "sloc": sloc,
        "d": d.astype(np.float64),
        "dl": (d + sl).astype(np.float64),
        "ln": sl.astype(np.float64),
        "T_cap": T_cap,
        "K_WIN": int(K_WIN),
        "bases": [int(b) for b in bases],
        "kcs": [int(k) for k in kcs],
    }


NGROUPS = G
SPLIT_WAITS = True


def _build(T_cap, K_WIN, bases, kcs, b3val):
    """Build the single SPMD Bass program (static; shared by all 8 cores)."""
    KC = K_WIN // 128
    T_pad = T_cap + 128
    nc = bass.Bass()

    def par(name, shape, dt):
        return nc.declare_dram_parameter(name, list(shape), dt, isOutput=False)

    statesT_p = par("statesT", [D, T_cap], BF)
    embedsT_p = par("embedsT", [D, T_cap], BF)
    dflat_p = par("dflat", [1, C], F16)
    deflat_p = par("deflat", [1, C], F16)
    lenflat_p = par("lenflat", [1, C], F16)
    aw1_p = par("aw1", [D, HID], BF)
    aw2_p = par("aw2", [HID, HID], BF)
    aw3_p = par("aw3m", [128, 8], BF)
    ab1_p = par("ab1m", [128, 8], F32)
    ab2_p = par("ab2m", [128, 8], F32)
    w1a_p = par("w1a", [D, HID], BF)
    w1b_p = par("w1b", [D, HID], BF)
    w1c_p = par("w1c", [D, HID], BF)
    w1d_p = par("w1d", [WD, HID], BF)
    wtT_p = par("wtT", [WD, LMAX], BF)
    b1r_p = par("b1r", [1, HID], BF)
    w2_p = par("w2", [HID, HID], BF)
    b2_p = par("b2m", [128, 8], F32)
    w3_p = par("w3m", [128, 8], BF)
    iotaCh_p = par("iotaCh", [128, KC], F32)
    ident_p = par("ident", [128, 128], BF)
    scores_p = nc.declare_dram_parameter("scores", [1, C], F32, isOutput=True)

    with PatchedTileContext(nc) as tc:
        with (
            tc.tile_pool(name="pp", bufs=1) as pp,
            tc.tile_pool(name="wst", bufs=2) as wst,
            tc.tile_pool(name="gp", bufs=2) as gp,
            tc.tile_pool(name="ps", bufs=2, space="PSUM") as ps,
            tc.tile_pool(name="dp", bufs=1, space="DRAM") as dp,
        ):
            dma = nc.sync.dma_start
            dmas = nc.scalar.dma_start

            # ---------- weights needed by block 0 go FIRST (PE warmup) -----
            def wload(param, tag_prefix):
                tiles = []
                for k in range(8):
                    t = pp.tile([128, HID], BF, name=f"{tag_prefix}{k}",
                                tag=f"{tag_prefix}{k}")
                    dma(out=t[:], in_=param[k * 128 : (k + 1) * 128, :])
                    tiles.append(t)
                return tiles

            ab1_t = pp.tile([128, 8], F32, name="ab1", tag="ab1")
            dma(out=ab1_t[:], in_=ab1_p[:])
            ab2_t = pp.tile([128, 8], F32, name="ab2", tag="ab2")
            dma(out=ab2_t[:], in_=ab2_p[:])
            aw3_t = pp.tile([128, 8], BF, name="aw3", tag="aw3")
            dma(out=aw3_t[:], in_=aw3_p[:])
            aw2_t = w1a_t = w1b_t = w1c_t = None

            b2_t = pp.tile([128, 8], F32, name="b2", tag="b2")
            dma(out=b2_t[:], in_=b2_p[:])
            w3_t = pp.tile([128, 8], BF, name="w3", tag="w3")
            dma(out=w3_t[:], in_=w3_p[:])
            b1r_t = pp.tile([1, HID], BF, name="b1r", tag="b1r")
            dmas(out=b1r_t[:], in_=b1r_p[:])
            w1d_t = pp.tile([WD, HID], BF, name="w1d", tag="w1d")
            dmas(out=w1d_t[:], in_=w1d_p[:])
            wtT_t = pp.tile([WD, 16], BF, name="wtT", tag="wtT")
            nc.vector.memset(wtT_t[:], 0.0)
            dmas(out=wtT_t[:, :LMAX], in_=wtT_p[:])
            ident_t = pp.tile([128, 128], BF, name="ident", tag="ident")
            dmas(out=ident_t[:], in_=ident_p[:])
            iotaCh_t = pp.tile([128, KC], F32, name="iotaCh", tag="iotaCh")
            dmas(out=iotaCh_t[:], in_=iotaCh_p[:])

            ones11 = pp.tile([1, 1], BF, name="ones11", tag="ones11")
            nc.vector.memset(ones11[:], 1.0)
            ones128 = pp.tile([1, 128], BF, name="ones128", tag="ones128")
            nc.vector.memset(ones128[:], 1.0)
            ones16_t = pp.tile([1, 16], BF, name="ones16", tag="ones16")
            nc.vector.memset(ones16_t[:], 1.0)

            ea_dram = dp.tile(
                [T_pad + 16], BF, name="ea_dram", tag="ea_dram")

            if NGROUPS < G:  # debug builds: ensure output is written
                zsc = pp.tile([1, C], F32, name="zsc", tag="zsc")
                nc.vector.memset(zsc[:], 0.0)
                dma(out=scores_p[:], in_=zsc[:])

            # ---------- P targets in DRAM [T_pad, HID] ----------
            P1d = dp.tile([T_pad, HID], BF, name="P1d", tag="P1d")
            P2d = dp.tile([T_pad, HID], BF, name="P2d", tag="P2d")
            P3d = dp.tile([T_pad, HID], BF, name="P3d", tag="P3d")
            Pd = (P1d, P2d, P3d)
            zrow = pp.tile([128, 512], BF, name="zrow", tag="zrow")
            nc.vector.memset(zrow[:], 0.0)

            # ---------- blocked token pipeline: attn MLP + P projections --
            ea_t = pp.tile([1, T_cap], BF, name="ea_t", tag="ea_t")
            _aw1_stage = []
            nblocks = [(n0, min(512, T_cap - n0)) for n0 in range(0, T_cap, 512)]

            for n0, nw in nblocks:
                sTw = []
                eTw = []
                for k in range(8):
                    if n0 == 0:
                        # interleave weight/data DMAs so the first L1 chain's
                        # operands arrive earliest
                        t = pp.tile([128, HID], BF, name=f"wA{k}",
                                    tag=f"wA{k}")
                        dma(out=t[:], in_=aw1_p[k * 128 : (k + 1) * 128, :])
                        _aw1_stage.append(t)
                    ts_ = wst.tile([128, 512], BF, name=f"sTw{k}", tag=f"sTw{k}")
                    dma(out=ts_[:, :nw],
                        in_=statesT_p[k * 128 : (k + 1) * 128, n0 : n0 + nw])
                    sTw.append(ts_)
                aw1_t = _aw1_stage
                for k in range(8):
                    te_ = wst.tile([128, 512], BF, name=f"eTw{k}", tag=f"eTw{k}",
                                   bufs=1)
                    dma(out=te_[:, :nw],
                        in_=embedsT_p[k * 128 : (k + 1) * 128, n0 : n0 + nw])
                    eTw.append(te_)
                if aw2_t is None:
                    aw2_t = wload(aw2_p, "wB")
                h1a = [wst.tile([128, 512], BF, name=f"h1a{h}", tag=f"h1a{h}",
                                bufs=1)
                       for h in range(8)]
                h2a = [wst.tile([128, 512], BF, name=f"h2a{h}", tag=f"h2a{h}",
                                bufs=1)
                       for h in range(8)]
                for hc in range(8):
                    pt = ps.tile([128, 512], F32, name="big", tag="big", bufs=2)
                    for k in range(8):
                        nc.tensor.matmul(
                            pt[:, :nw],
                            aw1_t[k][:, hc * 128 : (hc + 1) * 128],
                            sTw[k][:, :nw], start=(k == 0), stop=(k == 7))
                    nc.scalar.activation(
                        h1a[hc][:, :nw], pt[:, :nw], AF.Relu,
                        bias=ab1_t[:, hc : hc + 1])
                for hc in range(8):
                    pt = ps.tile([128, 512], F32, name="big", tag="big", bufs=2)
                    for k in range(8):
                        nc.tensor.matmul(
                            pt[:, :nw],
                            aw2_t[k][:, hc * 128 : (hc + 1) * 128],
                            h1a[k][:, :nw], start=(k == 0), stop=(k == 7))
                    nc.scalar.activation(
                        h2a[hc][:, :nw], pt[:, :nw], AF.Relu,
                        bias=ab2_t[:, hc : hc + 1])
                pt = ps.tile([1, 512], F32, name="big", tag="big", bufs=2)
                for k in range(8):
                    nc.tensor.matmul(
                        pt[:, :nw], aw3_t[:, k : k + 1], h2a[k][:, :nw],
                        start=(k == 0), stop=(k == 7))
                nc.scalar.activation(ea_t[:, n0 : n0 + nw], pt[:, :nw], AF.Exp)
                dmas(out=ea_dram[n0 : n0 + nw], in_=ea_t[0:1, n0 : n0 + nw])
                # per-128-token projections into DRAM
                if w1a_t is None:
                    w1a_t = wload(w1a_p, "wWA")
                    w1b_t = wload(w1b_p, "wWB")
                    w1c_t = wload(w1c_p, "wWC")

                def proj(pi, wt_, srcs, j, scale_ap=None):
                    js = slice(j * 128, (j + 1) * 128)
                    for h0 in (0, 512):
                        pt = ps.tile([128, 512], F32, name="big",
                                     tag="big", bufs=2)
                        for k in range(8):
                            nc.tensor.matmul(
                                pt[:], srcs[k][:, js],
                                wt_[k][:, h0 : h0 + 512],
                                start=(k == 0), stop=(k == 7))
                        stg = wst.tile([128, 512], BF, name=f"pstg{pi}",
                                       tag=f"pstg{pi}", bufs=1)
                        if scale_ap is None:
                            nc.scalar.copy(stg[:], pt[:])
                        else:
                            nc.scalar.mul(stg[:], pt[:], scale_ap)
                        dmas(out=Pd[pi][n0 + j * 128 : n0 + (j + 1) * 128,
                                        h0 : h0 + 512], in_=stg[:])

                nj = nw // 128
                for j in range(nj):
                    proj(0, w1a_t, sTw, j)
                    proj(1, w1b_t, sTw, j)
                # exp(attn) columns for this block: [1,128] rows -> [128,1]
                eac_sb = wst.tile([128, 4], F32, name="eacs", tag="eacs",
                                  bufs=2)
                eac_ps = ps.tile([128, 128], F32, name="eac", tag="rbc",
                                 bufs=1)
                for j in range(nj):
                    nc.tensor.matmul(
                        eac_ps[:, j : j + 1],
                        ea_t[0:1, n0 + j * 128 : n0 + (j + 1) * 128],
                        ones11[:], start=True, stop=True)
                nc.vector.tensor_copy(out=eac_sb[:, :nj], in_=eac_ps[:, :nj])
                for j in range(nj):
                    proj(2, w1c_t, eTw, j, scale_ap=eac_sb[:, j : j + 1])

            # pad P + ea beyond T_cap
            for pd in Pd:
                dma(out=pd[T_cap:, 0:512], in_=zrow[:])
                dma(out=pd[T_cap:, 512:1024], in_=zrow[:])
            zpad_t = pp.tile([1, 144], BF, name="zpad", tag="zpad")
            nc.vector.memset(zpad_t[:], 0.0)
            dma(out=ea_dram[T_cap:], in_=zpad_t[0:1, :])

            # span-index broadcasts (fp16, exact for ints < 2048)
            dbc = pp.tile([128, C], F16, name="dbc", tag="dbc")
            dma(out=dbc[:], in_=dflat_p[:].partition_broadcast(128))
            debc = pp.tile([128, C], F16, name="debc", tag="debc")
            dma(out=debc[:], in_=deflat_p[:].partition_broadcast(128))
            lnbc = pp.tile([16, C], F16, name="lnbc", tag="lnbc")
            dma(out=lnbc[:], in_=lenflat_p[:].partition_broadcast(16))

            # ---------- WB = width_table @ W1d + b1  → [16, HID] bf16 ------
            WB_t = pp.tile([16, HID], BF, name="WB", tag="WB")
            for h0 in range(0, HID, 512):
                pt = ps.tile([16, 512], F32, name="big", tag="big", bufs=2)
                nc.tensor.matmul(pt[:], wtT_t[:], w1d_t[:, h0 : h0 + 512],
                                 start=True, stop=False)
                nc.tensor.matmul(pt[:], ones16_t[:], b1r_t[:, h0 : h0 + 512],
                                 start=False, stop=True)
                nc.vector.tensor_copy(out=WB_t[:, h0 : h0 + 512], in_=pt[:])

            ea_h = ea_dram[:]            # AP to get the dram tensor handle

            # ---------- pre-pass: ssum + 1/ssum per group ----------
            rT_all = pp.tile([1, C], BF, name="rT_all", tag="rT_all")
            ea_all = pp.tile([128, G * KC], BF, name="ea_all", tag="ea_all")

            def build_band(g, kk):
                gs = slice(g * 128, (g + 1) * 128)
                tle = gp.tile([128, 128], F16, name="tle", tag="tle", bufs=1)
                nc.vector.tensor_scalar(
                    out=tle[:], in0=dbc[:, gs],
                    scalar1=iotaCh_t[:, kk : kk + 1], scalar2=None,
                    op0=AT.is_le)
                tge = gp.tile([128, 128], F16, name="tge", tag="tge", bufs=1)
                nc.vector.tensor_scalar(
                    out=tge[:], in0=debc[:, gs],
                    scalar1=iotaCh_t[:, kk : kk + 1], scalar2=None,
                    op0=AT.is_ge)
                band = gp.tile([128, 128], BF, name="band", tag="band", bufs=1)
                nc.vector.tensor_tensor(out=band[:], in0=tle[:], in1=tge[:],
                                        op=AT.mult)
                return band

            ssum_ps = ps.tile([128, 128], F32, name="ssum", tag="rbc",
                              bufs=1)
            for g in range(NGROUPS):
                KCg = kcs[g]
                dma(out=ea_all[:, g * KC : g * KC + KCg],
                     in_=AP(tensor=ea_h.tensor, offset=bases[g],
                            ap=[[1, 128], [128, KCg]]))
                for kk in range(KCg):
                    band = build_band(g, kk)
                    nc.tensor.matmul(
                        ssum_ps[:, g : g + 1], band[:],
                        ea_all[:, g * KC + kk : g * KC + kk + 1],
                        start=(kk == 0), stop=(kk == KCg - 1))
            rinv32 = gp.tile([128, G], F32, name="rinv32", tag="rinv32")
            nc.vector.reciprocal(rinv32[:], ssum_ps[:, :G])
            rinvbf = gp.tile([128, G], BF, name="rinvbf", tag="rinvbf")
            nc.vector.tensor_copy(out=rinvbf[:], in_=rinv32[:])
            rT8 = None
            for g in range(NGROUPS):
                if g % 8 == 0:
                    rT8 = ps.tile([1, 1024], BF, name="rT", tag="rT", bufs=1)
                c0 = (g % 8) * 128
                nc.tensor.transpose(rT8[:, c0 : c0 + 128],
                                    rinvbf[:, g : g + 1], ident_t[:])
                if g % 8 == 7:
                    nc.vector.tensor_copy(
                        out=rT_all[:, (g - 7) * 128 : (g + 1) * 128],
                        in_=rT8[:])

            # ---------- span groups ----------
            w2_t = wload(w2_p, "wA")     # reuse token-weight slots for L2

            def prep_group(g):
                """One-hots + softmax weight band for group g (runs one
                group ahead of the consuming h1 chain)."""
                KCg = kcs[g]
                gs = slice(g * 128, (g + 1) * 128)
                rbc_ps = ps.tile([128, 128], F32, name="rbc", tag="rbc",
                                 bufs=1)
                nc.tensor.matmul(rbc_ps[:], ones128[:], rT_all[0:1, gs],
                                 start=True, stop=True)
                ohs, ohe, wt = [], [], []
                for kk in range(KCg):
                    o1 = gp.tile([128, 128], BF, name=f"ohs{kk}",
                                 tag=f"ohs{kk}", bufs=2)
                    nc.vector.tensor_scalar(
                        out=o1[:], in0=dbc[:, gs],
                        scalar1=iotaCh_t[:, kk : kk + 1], scalar2=None,
                        op0=AT.is_equal)
                    ohs.append(o1)
                    o2 = gp.tile([128, 128], BF, name=f"ohe{kk}",
                                 tag=f"ohe{kk}", bufs=2)
                    nc.vector.tensor_scalar(
                        out=o2[:], in0=debc[:, gs],
                        scalar1=iotaCh_t[:, kk : kk + 1], scalar2=None,
                        op0=AT.is_equal)
                    ohe.append(o2)
                    band = build_band(g, kk)
                    w_ = gp.tile([128, 128], BF, name=f"wt{kk}",
                                 tag=f"wt{kk}", bufs=2)
                    nc.vector.tensor_tensor(out=w_[:], in0=band[:],
                                            in1=rbc_ps[:], op=AT.mult)
                    wt.append(w_)
                ohl = gp.tile([16, 128], BF, name="ohl", tag="ohl", bufs=2)
                nc.vector.tensor_scalar(
                    out=ohl[:], in0=lnbc[:, gs],
                    scalar1=iotaCh_t[:16, 0:1], scalar2=None,
                    op0=AT.is_equal)
                pw = []
                for pi in range(3):
                    tiles = []
                    for kk in range(KCg):
                        pt_ = wst.tile([128, HID], BF, name=f"pw{pi}_{kk}",
                                       tag=f"pw{pi}_{kk}", bufs=2)
                        r0 = bases[g] + kk * 128
                        dma(out=pt_[:], in_=Pd[pi][r0 : r0 + 128, :])
                        tiles.append(pt_)
                    pw.append(tiles)
                return (ohs, ohe, wt, ohl, pw)

            h1b = h2b = None
            prep = prep_group(0)
            for g in range(NGROUPS):
                KCg = kcs[g]
                if g % 4 == 0:
                    h1b = gp.tile([128, 8, 512], BF, name="h1b", tag="h1b",
                                  bufs=1)
                gcol = (g % 4) * 128

                ohs, ohe, wt, ohl, pw = prep
                if g + 1 < NGROUPS:
                    prep = prep_group(g + 1)

                # h1[n, h]: one-hot stationary, P moving (dense M=512 MMs)
                hp = ps.tile([128, 1024], F32, name="h1f", tag="h1f",
                             bufs=2)
                steps = []
                for kk in range(KCg):
                    steps.append((ohs[kk][:], pw[0][kk]))
                    steps.append((ohe[kk][:], pw[1][kk]))
                steps.append((ohl[:], None))
                for kk in range(KCg):
                    steps.append((wt[kk][:], pw[2][kk]))
                ns = len(steps)
                for h0 in (0, 512):
                    for i, (lhsT, rhs_t) in enumerate(steps):
                        rhs = (WB_t[:, h0 : h0 + 512] if rhs_t is None
                               else rhs_t[:, h0 : h0 + 512])
                        nc.tensor.matmul(
                            hp[:, h0 : h0 + 512], lhsT, rhs,
                            start=(i == 0), stop=(i == ns - 1))
                h1s = gp.tile([128, 1024], BF, name="h1s", tag="h1s",
                              bufs=2)
                nc.vector.tensor_scalar(
                    out=h1s[:], in0=hp[:], scalar1=0.0, scalar2=None,
                    op0=AT.max)
                for k in range(8):
                    nc.scalar.dma_start_transpose(
                        out=h1b[:, k, gcol : gcol + 128],
                        in_=h1s[:, k * 128 : (k + 1) * 128])

                # every 4 groups: span-MLP L2+L3 on the 512-col block
                if g % 4 == 3:
                    b0 = (g // 4) * 512
                    h2b = [gp.tile([128, 512], BF, name=f"h2b{k}",
                                   tag=f"h2b{k}", bufs=1)
                           for k in range(8)]
                    for h2c in range(8):
                        pt = ps.tile([128, 512], F32, name="big", tag="big",
                                     bufs=2)
                        for k in range(8):
                            nc.tensor.matmul(
                                pt[:], w2_t[k][:, h2c * 128 : (h2c + 1) * 128],
                                h1b[:, k, :], start=(k == 0), stop=(k == 7))
                        nc.vector.tensor_scalar(
                            out=h2b[h2c][:], in0=pt[:],
                            scalar1=b2_t[:, h2c : h2c + 1], scalar2=0.0,
                            op0=AT.add, op1=AT.max)
                    pt = ps.tile([1, 512], F32, name="big", tag="big", bufs=2)
                    for k in range(8):
                        nc.tensor.matmul(pt[:], w3_t[:, k : k + 1], h2b[k][:],
                                         start=(k == 0), stop=(k == 7))
                    ob = gp.tile([1, 512], F32, name="ob", tag="ob")
                    nc.vector.tensor_scalar(out=ob[:], in0=pt[:],
                                            scalar1=float(b3val), scalar2=None,
                                            op0=AT.add)
                    dma(out=scores_p[:, b0 : b0 + 512], in_=ob[:])

    if SPLIT_WAITS:
        _split_waits(nc)
    return nc



def _split_waits(nc, max_waits=1):
    """This walrus build rejects instructions carrying >max_waits sem waits
    ("Too many sync wait commands"). Hoist excess waits onto same-engine
    NoOps placed immediately before the instruction — identical semantics
    (engine queues are in-order)."""
    ctr = [0]
    for f in nc.m.functions:
        for blk in f.blocks:
            out = []
            for ins in blk.instructions:
                si = getattr(ins, "sync_info", None)
                if si is not None and si.on_wait and len(si.on_wait) > max_waits:
                    waits = list(si.on_wait)
                    for w in waits[:-max_waits]:
                        ctr[0] += 1
                        nop = mybir.InstNoOp(
                            name=f"I-wsplit-{ctr[0]}", ins=[], outs=[],
                            sync_info=mybir.SyncInfo(on_wait=[w], on_update=[]),
                        )
                        nop.engine = ins.engine
                        out.append(nop)
                    ins.sync_info = mybir.SyncInfo(
                        on_wait=waits[-max_waits:],
                        on_update=list(si.on_update or []),
                    )
                out.append(ins)
            blk.instructions[:] = out
    return ctr[0]


_CACHE = {}
LAST_EXEC_NS = None
TRACE = False


def _install_ntff_shim():
    try:
        import antenv.axon_hooks  # noqa: F401
        return
    except ImportError:
        pass
    try:
        from trn_agent_boot.trn_boot import _ntff_profile_via_ctypes
        hook = _ntff_profile_via_ctypes("/opt/axon/libaxon_pjrt.so")
    except Exception:
        hook = None
    m1 = types.ModuleType("antenv")
    m2 = types.ModuleType("antenv.axon_hooks")
    m2.get_axon_ntff_profile_hook = lambda: hook
    m2.set_axon_ntff_profile_hook = lambda h: None
    m1.axon_hooks = m2
    sys.modules.setdefault("antenv", m1)
    sys.modules["antenv.axon_hooks"] = m2


def _prepare(inputs):
    inp = {k: np.asarray(v) for k, v in inputs.items()}
    ss = inp["span_starts"].astype(np.int64)
    sl = inp["span_lengths"].astype(np.int64)
    plan = _plan(ss, sl)
    T_cap, K_WIN, bases = plan["T_cap"], plan["K_WIN"], plan["bases"]
    KC = K_WIN // 128
    b3val = float(np.asarray(inp["score_b3"]).reshape(-1)[0])

    kcs = plan["kcs"]
    key = (T_cap, K_WIN, tuple(bases), tuple(kcs), b3val)
    if key not in _CACHE:
        _CACHE[key] = _build(T_cap, K_WIN, bases, kcs, b3val)
    nc = _CACHE[key]

    def bfc(x):
        return np.ascontiguousarray(np.asarray(x, dtype=np.float32)).astype(bf16)

    sw1 = inp["score_w1"].astype(np.float32)
    shared = {
        "aw1": bfc(inp["attn_w1"]),
        "aw2": bfc(inp["attn_w2"]),
        "aw3m": bfc(inp["attn_w3"].reshape(8, 128).T),
        "ab1m": np.ascontiguousarray(
            inp["attn_b1"].astype(np.float32).reshape(8, 128).T),
        "ab2m": np.ascontiguousarray(
            inp["attn_b2"].astype(np.float32).reshape(8, 128).T),
        "w1a": bfc(sw1[0:1024]),
        "w1b": bfc(sw1[1024:2048]),
        "w1c": bfc(sw1[2048:3072]),
        "w1d": bfc(sw1[3072:3092]),
        "wtT": bfc(inp["width_table"].T),
        "b1r": bfc(inp["score_b1"].reshape(1, HID)),
        "w2": bfc(inp["score_w2"]),
        "b2m": np.ascontiguousarray(
            inp["score_b2"].astype(np.float32).reshape(8, 128).T),
        "w3m": bfc(inp["score_w3"].reshape(8, 128).T),
        "iotaCh": np.ascontiguousarray(
            (np.arange(128, dtype=np.float32)[:, None]
             + 128.0 * np.arange(KC, dtype=np.float32)[None, :])
        ),
        "ident": np.eye(128, dtype=np.float32).astype(bf16),
    }

    states = inp["states"].astype(np.float32)
    embeds = inp["embeds"].astype(np.float32)
    in_maps = []
    for c in range(N_CORES):
        cb = int(plan["core_base"][c])
        stl = np.zeros((T_cap, D), np.float32)
        eml = np.zeros((T_cap, D), np.float32)
        hi = min(T, cb + T_cap)
        stl[: hi - cb] = states[cb:hi]
        eml[: hi - cb] = embeds[cb:hi]
        m = dict(shared)
        m["statesT"] = np.ascontiguousarray(stl.T).astype(bf16)
        m["embedsT"] = np.ascontiguousarray(eml.T).astype(bf16)
        d = plan["d"][c]
        dl = plan["dl"][c]
        ln = plan["ln"][c]
        m["dflat"] = d.reshape(1, C).astype(np.float16)
        m["deflat"] = dl.reshape(1, C).astype(np.float16)
        m["lenflat"] = ln.reshape(1, C).astype(np.float16)
        in_maps.append(m)

    return nc, in_maps, plan


def kernel(**inputs):
    global LAST_EXEC_NS
    from concourse.bass_utils import run_bass_kernel_spmd

    nc, in_maps, plan = _prepare(inputs)
    _install_ntff_shim()
    res = run_bass_kernel_spmd(nc, in_maps, list(range(N_CORES)), trace=TRACE)
    LAST_EXEC_NS = res.exec_time_ns

    out = np.empty(NSPAN, np.float32)
    for c in range(N_CORES):
        out[plan["order"][c * C : (c + 1) * C]] = np.asarray(
            res.results[c]["scores"]).reshape(-1)
    return out.reshape(NSPAN, 1)


# revision 11
# speedup vs baseline: 1.7299x; 1.7299x over previous
"""Trainium2 Bass kernel for nn_MentionScore.

Strategy: sort spans by start, shard 2048 consecutive sorted spans per core.
Each core only touches a ~1.1k-token window of states/embeds (host passes the
window pre-transposed, bf16). The ragged gather/softmax/weighted-sum becomes
dense matmuls against one-hot / banded matrices built on-device with
iota-compare tensor ops. Layer-1 of the span MLP is algebraically folded:
  h1 = relu(OH_s.T@P1 + OH_e.T@P2 + W.T@P3' + onehot(len).T@WB)
with P1=states@W1a, P2=states@W1b, P3'=exp(attn)*(embeds@W1c) per token and
WB = width_table@W1d + b1.

Softmax separability: exp(span_attns[n,l]) = exp(attns[start_n+l]) means the
softmax numerator matrix factors as diag(exp(attns)) @ Band where Band is the
0/1 in-range indicator. exp() is folded into P3 per token (P3'), the
denominator ssum = Band.T @ exp(attns) comes from an N=1 matmul riding the
Band stationary, and the weight band is W = Band * broadcast(1/ssum), with the
broadcast done by a K=1 PE outer product.
"""

import sys
import types

import numpy as np
import ml_dtypes

import concourse.bass as bass
import concourse.mybir as mybir
from concourse.ap import AP
from concourse.tile import TileContext
from concourse.vector_clock import ScopedClock

BF = mybir.dt.bfloat16
F16 = mybir.dt.float16
F32 = mybir.dt.float32
AT = mybir.AluOpType
AF = mybir.ActivationFunctionType
AX = mybir.AxisListType
bf16 = ml_dtypes.bfloat16

N_CORES = 8
T, NSPAN, D, HID, LMAX, WD = 8192, 16384, 1024, 1024, 10, 20
C = NSPAN // N_CORES          # spans per core
G = C // 128                  # 128-span groups per core


class PatchedTileContext(TileContext):
    """Workaround: walrus rejects the tail Drain when it carries >1 sem wait
    ("Too many sync wait commands"). Put each wait on its own NoOp instead."""

    def _drain_and_barrier(self, tick_clock, wait_clock):
        nc = self.nc
        drain_inst = nc.sync.drain()
        wait_clock.add_sem_waits(
            drain_inst.ins, ScopedClock({None: tick_clock.global_clock})
        )
        si = drain_inst.ins.sync_info
        if si is not None and si.on_wait is not None and len(si.on_wait) > 1:
            waits = list(si.on_wait)
            drain_inst.ins.sync_info = mybir.SyncInfo(
                on_wait=[waits[0]], on_update=list(si.on_update or [])
            )
            for w in waits[1:]:
                nop = nc.sync.nop()
                nop.ins.sync_info = mybir.SyncInfo(on_wait=[w], on_update=[])

        nc.all_engine_barrier()
        assert self.sems is not None
        popped = nc._tile_sem_poison_stack.pop()
        assert popped is self._sem_poison
        nc.clear_and_free_semaphores(list(self.sems.allocated().values()))
        nc.all_engine_barrier()


def _ceil128(x):
    return int(-(-int(x) // 128) * 128)


def _plan(span_starts, span_lengths):
    """Host-side sharding plan. Returns per-core data + static layout consts."""
    order = np.argsort(span_starts, kind="stable").astype(np.int64)
    ss = span_starts[order].reshape(N_CORES, C).astype(np.int64)
    sl = span_lengths[order].reshape(N_CORES, C).astype(np.int64)
    core_base = ss[:, 0].copy()
    sloc = ss - core_base[:, None]
    eloc = sloc + sl

    T_cap = _ceil128(int(eloc.max()) + 1)
    # unaligned, shared-across-cores group window bases + per-group k-tiles
    mn = sloc[:, ::128].min(axis=0)                       # [G]
    mx = eloc.reshape(N_CORES, G, 128).max(axis=2).max(axis=0)  # [G]
    need = mx - mn + 1
    kcs = np.maximum((need + 127) // 128, 1)
    T_pad = T_cap + 128
    bases = mn.copy()
    for _ in range(3):
        bases = np.minimum(mn, T_pad - kcs * 128)
        bad = (mx - bases + 1) > kcs * 128
        if not bad.any():
            break
        kcs[bad] += 1
    K_WIN = int(kcs.max()) * 128
    d = sloc - np.repeat(bases, 128)[None, :]
    assert d.min() >= 0 and ((d + sl).reshape(N_CORES, G, 128).max(axis=2)
                             <= kcs[None, :] * 128 - 1).all(), "window overflow"

    return {
        "order": order,
        "core_base": core_base,
        "sloc": sloc,
        "d": d.astype(np.float64),
        "dl": (d + sl).astype(np.float64),
        "ln": sl.astype(np.float64),
        "T_cap": T_cap,
        "K_WIN": int(K_WIN),
        "bases": [int(b) for b in bases],
        "kcs": [int(k) for k in kcs],
    }


NGROUPS = G
SPLIT_WAITS = True


def _build(T_cap, K_WIN, bases, kcs, b3val):
    """Build the single SPMD Bass program (static; shared by all 8 cores)."""
    KC = K_WIN // 128
    T_pad = T_cap + 128
    nc = bass.Bass()

    def par(name, shape, dt):
        return nc.declare_dram_parameter(name, list(shape), dt, isOutput=False)

    statesT_p = par("statesT", [D, T_cap], BF)
    embedsT_p = par("embedsT", [D, T_cap], BF)
    dflat_p = par("dflat", [1, C], F16)
    deflat_p = par("deflat", [1, C], F16)
    lenflat_p = par("lenflat", [1, C], F16)
    aw1_p = par("aw1", [D, HID], BF)
    aw2_p = par("aw2", [HID, HID], BF)
    aw3_p = par("aw3m", [128, 8], BF)
    ab1_p = par("ab1m", [128, 8], F32)
    ab2_p = par("ab2m", [128, 8], F32)
    w1a_p = par("w1a", [D, HID], BF)
    w1b_p = par("w1b", [D, HID], BF)
    w1c_p = par("w1c", [D, HID], BF)
    w1d_p = par("w1d", [WD, HID], BF)
    wtT_p = par("wtT", [WD, LMAX], BF)
    b1r_p = par("b1r", [1, HID], BF)
    w2_p = par("w2", [HID, HID], BF)
    b2_p = par("b2m", [128, 8], F32)
    w3_p = par("w3m", [128, 8], BF)
    iotaCh_p = par("iotaCh", [128, KC], F32)
    ident_p = par("ident", [128, 128], BF)
    scores_p = nc.declare_dram_parameter("scores", [1, C], F32, isOutput=True)

    with PatchedTileContext(nc) as tc:
        with (
            tc.tile_pool(name="pp", bufs=1) as pp,
            tc.tile_pool(name="wst", bufs=2) as wst,
            tc.tile_pool(name="gp", bufs=2) as gp,
            tc.tile_pool(name="ps", bufs=2, space="PSUM") as ps,
            tc.tile_pool(name="dp", bufs=1, space="DRAM") as dp,
        ):
            dma = nc.sync.dma_start
            dmas = nc.scalar.dma_start

            # ---------- weights needed by block 0 go FIRST (PE warmup) -----
            def wload(param, tag_prefix):
                tiles = []
                for k in range(8):
                    t = pp.tile([128, HID], BF, name=f"{tag_prefix}{k}",
                                tag=f"{tag_prefix}{k}")
                    dma(out=t[:], in_=param[k * 128 : (k + 1) * 128, :])
                    tiles.append(t)
                return tiles

            ab1_t = pp.tile([128, 8], F32, name="ab1", tag="ab1")
            dma(out=ab1_t[:], in_=ab1_p[:])
            ab2_t = pp.tile([128, 8], F32, name="ab2", tag="ab2")
            dma(out=ab2_t[:], in_=ab2_p[:])
            aw3_t = pp.tile([128, 8], BF, name="aw3", tag="aw3")
            dma(out=aw3_t[:], in_=aw3_p[:])
            aw2_t = w1a_t = w1b_t = w1c_t = None

            b2_t = pp.tile([128, 8], F32, name="b2", tag="b2")
            dma(out=b2_t[:], in_=b2_p[:])
            w3_t = pp.tile([128, 8], BF, name="w3", tag="w3")
            dma(out=w3_t[:], in_=w3_p[:])
            b1r_t = pp.tile([1, HID], BF, name="b1r", tag="b1r")
            dmas(out=b1r_t[:], in_=b1r_p[:])
            w1d_t = pp.tile([WD, HID], BF, name="w1d", tag="w1d")
            dmas(out=w1d_t[:], in_=w1d_p[:])
            wtT_t = pp.tile([WD, 16], BF, name="wtT", tag="wtT")
            nc.vector.memset(wtT_t[:], 0.0)
            dmas(out=wtT_t[:, :LMAX], in_=wtT_p[:])
            ident_t = pp.tile([128, 128], BF, name="ident", tag="ident")
            dmas(out=ident_t[:], in_=ident_p[:])
            iotaCh_t = pp.tile([128, KC], F32, name="iotaCh", tag="iotaCh")
            dmas(out=iotaCh_t[:], in_=iotaCh_p[:])

            ones11 = pp.tile([1, 1], BF, name="ones11", tag="ones11")
            nc.vector.memset(ones11[:], 1.0)
            ones128 = pp.tile([1, 128], BF, name="ones128", tag="ones128")
            nc.vector.memset(ones128[:], 1.0)
            ones16_t = pp.tile([1, 16], BF, name="ones16", tag="ones16")
            nc.vector.memset(ones16_t[:], 1.0)

            ea_dram = dp.tile(
                [T_pad + 16], BF, name="ea_dram", tag="ea_dram")

            if NGROUPS < G:  # debug builds: ensure output is written
                zsc = pp.tile([1, C], F32, name="zsc", tag="zsc")
                nc.vector.memset(zsc[:], 0.0)
                dma(out=scores_p[:], in_=zsc[:])

            # ---------- P targets in DRAM [T_pad, HID] ----------
            P1d = dp.tile([T_pad, HID], BF, name="P1d", tag="P1d")
            P2d = dp.tile([T_pad, HID], BF, name="P2d", tag="P2d")
            P3d = dp.tile([T_pad, HID], BF, name="P3d", tag="P3d")
            Pd = (P1d, P2d, P3d)
            zrow = pp.tile([128, 512], BF, name="zrow", tag="zrow")
            nc.vector.memset(zrow[:], 0.0)

            # ---------- blocked token pipeline: attn MLP + P projections --
            ea_t = pp.tile([1, T_cap], BF, name="ea_t", tag="ea_t")
            _aw1_stage = []
            nblocks = [(n0, min(512, T_cap - n0)) for n0 in range(0, T_cap, 512)]

            for n0, nw in nblocks:
                sTw = []
                eTw = []
                for k in range(8):
                    if n0 == 0:
                        # interleave weight/data DMAs so the first L1 chain's
                        # operands arrive earliest
                        t = pp.tile([128, HID], BF, name=f"wA{k}",
                                    tag=f"wA{k}")
                        dma(out=t[:], in_=aw1_p[k * 128 : (k + 1) * 128, :])
                        _aw1_stage.append(t)
                    ts_ = wst.tile([128, 512], BF, name=f"sTw{k}", tag=f"sTw{k}")
                    dma(out=ts_[:, :nw],
                        in_=statesT_p[k * 128 : (k + 1) * 128, n0 : n0 + nw])
                    sTw.append(ts_)
                aw1_t = _aw1_stage
                for k in range(8):
                    te_ = wst.tile([128, 512], BF, name=f"eTw{k}", tag=f"eTw{k}",
                                   bufs=1)
                    dma(out=te_[:, :nw],
                        in_=embedsT_p[k * 128 : (k + 1) * 128, n0 : n0 + nw])
                    eTw.append(te_)
                if aw2_t is None:
                    aw2_t = wload(aw2_p, "wB")
                h1a = [wst.tile([128, 512], BF, name=f"h1a{h}", tag=f"h1a{h}",
                                bufs=1)
                       for h in range(8)]
                h2a = [wst.tile([128, 512], BF, name=f"h2a{h}", tag=f"h2a{h}",
                                bufs=1)
                       for h in range(8)]
                for hc in range(8):
                    pt = ps.tile([128, 512], F32, name="big", tag="big", bufs=2)
                    for k in range(8):
                        nc.tensor.matmul(
                            pt[:, :nw],
                            aw1_t[k][:, hc * 128 : (hc + 1) * 128],
                            sTw[k][:, :nw], start=(k == 0), stop=(k == 7))
                    nc.scalar.activation(
                        h1a[hc][:, :nw], pt[:, :nw], AF.Relu,
                        bias=ab1_t[:, hc : hc + 1])
                for hc in range(8):
                    pt = ps.tile([128, 512], F32, name="big", tag="big", bufs=2)
                    for k in range(8):
                        nc.tensor.matmul(
                            pt[:, :nw],
                            aw2_t[k][:, hc * 128 : (hc + 1) * 128],
                            h1a[k][:, :nw], start=(k == 0), stop=(k == 7))
                    nc.scalar.activation(
                        h2a[hc][:, :nw], pt[:, :nw], AF.Relu,
                        bias=ab2_t[:, hc : hc + 1])
                pt = ps.tile([1, 512], F32, name="big", tag="big", bufs=2)
                for k in range(8):
                    nc.tensor.matmul(
                        pt[:, :nw], aw3_t[:, k : k + 1], h2a[k][:, :nw],
                        start=(k == 0), stop=(k == 7))
                nc.scalar.activation(ea_t[:, n0 : n0 + nw], pt[:, :nw], AF.Exp)
                dmas(out=ea_dram[n0 : n0 + nw], in_=ea_t[0:1, n0 : n0 + nw])
                # per-128-token projections into DRAM
                if w1a_t is None:
                    w1a_t = wload(w1a_p, "wWA")
                    w1b_t = wload(w1b_p, "wWB")
                    w1c_t = wload(w1c_p, "wWC")

                def proj(pi, wt_, srcs, j, scale_ap=None):
                    js = slice(j * 128, (j + 1) * 128)
                    for h0 in (0, 512):
                        pt = ps.tile([128, 512], F32, name="big",
                                     tag="big", bufs=2)
                        for k in range(8):
                            nc.tensor.matmul(
                                pt[:], srcs[k][:, js],
                                wt_[k][:, h0 : h0 + 512],
                                start=(k == 0), stop=(k == 7))
                        stg = wst.tile([128, 512], BF, name=f"pstg{pi}",
                                       tag=f"pstg{pi}", bufs=1)
                        if scale_ap is None:
                            nc.scalar.copy(stg[:], pt[:])
                        else:
                            nc.scalar.mul(stg[:], pt[:], scale_ap)
                        dmas(out=Pd[pi][n0 + j * 128 : n0 + (j + 1) * 128,
                                        h0 : h0 + 512], in_=stg[:])

                nj = nw // 128
                for j in range(nj):
                    proj(0, w1a_t, sTw, j)
                    proj(1, w1b_t, sTw, j)
                # exp(attn) columns for this block: [1,128] rows -> [128,1]
                eac_sb = wst.tile([128, 4], F32, name="eacs", tag="eacs",
                                  bufs=2)
                eac_ps = ps.tile([128, 128], F32, name="eac", tag="rbc",
                                 bufs=1)
                for j in range(nj):
                    nc.tensor.matmul(
                        eac_ps[:, j : j + 1],
                        ea_t[0:1, n0 + j * 128 : n0 + (j + 1) * 128],
                        ones11[:], start=True, stop=True)
                nc.vector.tensor_copy(out=eac_sb[:, :nj], in_=eac_ps[:, :nj])
                for j in range(nj):
                    proj(2, w1c_t, eTw, j, scale_ap=eac_sb[:, j : j + 1])

            # pad P + ea beyond T_cap
            for pd in Pd:
                dma(out=pd[T_cap:, 0:512], in_=zrow[:])
                dma(out=pd[T_cap:, 512:1024], in_=zrow[:])
            zpad_t = pp.tile([1, 144], BF, name="zpad", tag="zpad")
            nc.vector.memset(zpad_t[:], 0.0)
            dma(out=ea_dram[T_cap:], in_=zpad_t[0:1, :])

            # span-index broadcasts (fp16, exact for ints < 2048)
            dbc = pp.tile([128, C], F16, name="dbc", tag="dbc")
            dma(out=dbc[:], in_=dflat_p[:].partition_broadcast(128))
            debc = pp.tile([128, C], F16, name="debc", tag="debc")
            dma(out=debc[:], in_=deflat_p[:].partition_broadcast(128))
            lnbc = pp.tile([16, C], F16, name="lnbc", tag="lnbc")
            dma(out=lnbc[:], in_=lenflat_p[:].partition_broadcast(16))

            # ---------- WB = width_table @ W1d + b1  → [16, HID] bf16 ------
            WB_t = pp.tile([16, HID], BF, name="WB", tag="WB")
            for h0 in range(0, HID, 512):
                pt = ps.tile([16, 512], F32, name="big", tag="big", bufs=2)
                nc.tensor.matmul(pt[:], wtT_t[:], w1d_t[:, h0 : h0 + 512],
                                 start=True, stop=False)
                nc.tensor.matmul(pt[:], ones16_t[:], b1r_t[:, h0 : h0 + 512],
                                 start=False, stop=True)
                nc.vector.tensor_copy(out=WB_t[:, h0 : h0 + 512], in_=pt[:])

            ea_h = ea_dram[:]            # AP to get the dram tensor handle

            # ---------- pre-pass: ssum + 1/ssum per group ----------
            rT_all = pp.tile([1, C], BF, name="rT_all", tag="rT_all")
            ea_all = pp.tile([128, G * KC], BF, name="ea_all", tag="ea_all")

            def build_band(g, kk):
                gs = slice(g * 128, (g + 1) * 128)
                tle = gp.tile([128, 128], F16, name="tle", tag="tle", bufs=1)
                nc.vector.tensor_scalar(
                    out=tle[:], in0=dbc[:, gs],
                    scalar1=iotaCh_t[:, kk : kk + 1], scalar2=None,
                    op0=AT.is_le)
                tge = gp.tile([128, 128], F16, name="tge", tag="tge", bufs=1)
                nc.vector.tensor_scalar(
                    out=tge[:], in0=debc[:, gs],
                    scalar1=iotaCh_t[:, kk : kk + 1], scalar2=None,
                    op0=AT.is_ge)
                band = gp.tile([128, 128], BF, name="band", tag="band", bufs=1)
                nc.vector.tensor_tensor(out=band[:], in0=tle[:], in1=tge[:],
                                        op=AT.mult)
                return band

            ssum_ps = ps.tile([128, 128], F32, name="ssum", tag="rbc",
                              bufs=1)
            for g in range(NGROUPS):
                KCg = kcs[g]
                dma(out=ea_all[:, g * KC : g * KC + KCg],
                     in_=AP(tensor=ea_h.tensor, offset=bases[g],
                            ap=[[1, 128], [128, KCg]]))
                for kk in range(KCg):
                    band = build_band(g, kk)
                    nc.tensor.matmul(
                        ssum_ps[:, g : g + 1], band[:],
                        ea_all[:, g * KC + kk : g * KC + kk + 1],
                        start=(kk == 0), stop=(kk == KCg - 1))
            rinv32 = gp.tile([128, G], F32, name="rinv32", tag="rinv32")
            nc.vector.reciprocal(rinv32[:], ssum_ps[:, :G])
            rinvbf = gp.tile([128, G], BF, name="rinvbf", tag="rinvbf")
            nc.vector.tensor_copy(out=rinvbf[:], in_=rinv32[:])
            rT8 = None
            for g in range(NGROUPS):
                if g % 8 == 0:
                    rT8 = ps.tile([128, 8, 128], BF, name="tr", tag="tr",
                                  bufs=1)
                nc.tensor.transpose(rT8[0:1, g % 8, :],
                                    rinvbf[:, g : g + 1], ident_t[:])
                if g % 8 == 7:
                    nc.vector.tensor_copy(
                        out=rT_all[:, (g - 7) * 128 : (g + 1) * 128],
                        in_=rT8[0:1, :, :])

            # ---------- span groups ----------
            w2_t = wload(w2_p, "wA")     # reuse token-weight slots for L2

            def prep_group(g):
                """One-hots + softmax weight band for group g (runs one
                group ahead of the consuming h1 chain)."""
                KCg = kcs[g]
                gs = slice(g * 128, (g + 1) * 128)
                rbc_ps = ps.tile([128, 128], F32, name="rbc", tag="rbc",
                                 bufs=1)
                nc.tensor.matmul(rbc_ps[:], ones128[:], rT_all[0:1, gs],
                                 start=True, stop=True)
                ohs, ohe, wt = [], [], []
                for kk in range(KCg):
                    o1 = gp.tile([128, 128], BF, name=f"ohs{kk}",
                                 tag=f"ohs{kk}", bufs=2)
                    nc.vector.tensor_scalar(
                        out=o1[:], in0=dbc[:, gs],
                        scalar1=iotaCh_t[:, kk : kk + 1], scalar2=None,
                        op0=AT.is_equal)
                    ohs.append(o1)
                    o2 = gp.tile([128, 128], BF, name=f"ohe{kk}",
                                 tag=f"ohe{kk}", bufs=2)
                    nc.vector.tensor_scalar(
                        out=o2[:], in0=debc[:, gs],
                        scalar1=iotaCh_t[:, kk : kk + 1], scalar2=None,
                        op0=AT.is_equal)
                    ohe.append(o2)
                    band = build_band(g, kk)
                    w_ = gp.tile([128, 128], BF, name=f"wt{kk}",
                                 tag=f"wt{kk}", bufs=2)
                    nc.vector.tensor_tensor(out=w_[:], in0=band[:],
                                            in1=rbc_ps[:], op=AT.mult)
                    wt.append(w_)
                ohl = gp.tile([16, 128], BF, name="ohl", tag="ohl", bufs=2)
                nc.vector.tensor_scalar(
                    out=ohl[:], in0=lnbc[:, gs],
                    scalar1=iotaCh_t[:16, 0:1], scalar2=None,
                    op0=AT.is_equal)
                pw = []
                for pi in range(3):
                    tiles = []
                    for kk in range(KCg):
                        pt_ = wst.tile([128, HID], BF, name=f"pw{pi}_{kk}",
                                       tag=f"pw{pi}_{kk}", bufs=2)
                        r0 = bases[g] + kk * 128
                        dma(out=pt_[:], in_=Pd[pi][r0 : r0 + 128, :])
                        tiles.append(pt_)
                    pw.append(tiles)
                return (ohs, ohe, wt, ohl, pw)

            h1b = h2b = None
            prep = prep_group(0)
            for g in range(NGROUPS):
                KCg = kcs[g]
                if g % 4 == 0:
                    h1b = gp.tile([128, 8, 512], BF, name="h1b", tag="h1b",
                                  bufs=1)
                gcol = (g % 4) * 128

                ohs, ohe, wt, ohl, pw = prep
                if g + 1 < NGROUPS:
                    prep = prep_group(g + 1)

                # h1[n, h]: one-hot stationary, P moving (dense M=512 MMs)
                hp = ps.tile([128, 1024], F32, name="h1f", tag="h1f",
                             bufs=2)
                steps = []
                for kk in range(KCg):
                    steps.append((ohs[kk][:], pw[0][kk]))
                    steps.append((ohe[kk][:], pw[1][kk]))
                steps.append((ohl[:], None))
                for kk in range(KCg):
                    steps.append((wt[kk][:], pw[2][kk]))
                ns = len(steps)
                for h0 in (0, 512):
                    for i, (lhsT, rhs_t) in enumerate(steps):
                        rhs = (WB_t[:, h0 : h0 + 512] if rhs_t is None
                               else rhs_t[:, h0 : h0 + 512])
                        nc.tensor.matmul(
                            hp[:, h0 : h0 + 512], lhsT, rhs,
                            start=(i == 0), stop=(i == ns - 1))
                h1s = gp.tile([128, 1024], BF, name="h1s", tag="h1s",
                              bufs=2)
                nc.vector.tensor_scalar(
                    out=h1s[:], in0=hp[:], scalar1=0.0, scalar2=None,
                    op0=AT.max)
                tr = ps.tile([128, 8, 128], BF, name="tr", tag="tr",
                             bufs=1)
                for k in range(8):
                    nc.tensor.transpose(
                        tr[:, k, :], h1s[:, k * 128 : (k + 1) * 128],
                        ident_t[:])
                nc.vector.tensor_copy(
                    out=h1b[:, :, gcol : gcol + 128], in_=tr[:])

                # every 4 groups: span-MLP L2+L3 on the 512-col block
                if g % 4 == 3:
                    b0 = (g // 4) * 512
                    h2b = [gp.tile([128, 512], BF, name=f"h2b{k}",
                                   tag=f"h2b{k}", bufs=1)
                           for k in range(8)]
                    for h2c in range(8):
                        pt = ps.tile([128, 512], F32, name="big", tag="big",
                                     bufs=2)
                        for k in range(8):
                            nc.tensor.matmul(
                                pt[:], w2_t[k][:, h2c * 128 : (h2c + 1) * 128],
                                h1b[:, k, :], start=(k == 0), stop=(k == 7))
                        nc.vector.tensor_scalar(
                            out=h2b[h2c][:], in0=pt[:],
                            scalar1=b2_t[:, h2c : h2c + 1], scalar2=0.0,
                            op0=AT.add, op1=AT.max)
                    pt = ps.tile([1, 512], F32, name="big", tag="big", bufs=2)
                    for k in range(8):
                        nc.tensor.matmul(pt[:], w3_t[:, k : k + 1], h2b[k][:],
                                         start=(k == 0), stop=(k == 7))
                    ob = gp.tile([1, 512], F32, name="ob", tag="ob")
                    nc.vector.tensor_scalar(out=ob[:], in0=pt[:],
                                            scalar1=float(b3val), scalar2=None,
                                            op0=AT.add)
                    dma(out=scores_p[:, b0 : b0 + 512], in_=ob[:])

    if SPLIT_WAITS:
        _split_waits(nc)
    return nc



def _split_waits(nc, max_waits=1):
    """This walrus build rejects instructions carrying >max_waits sem waits
    ("Too many sync wait commands"). Hoist excess waits onto same-engine
    NoOps placed immediately before the instruction — identical semantics
    (engine queues are in-order)."""
    ctr = [0]
    for f in nc.m.functions:
        for blk in f.blocks:
            out = []
            for ins in blk.instructions:
                si = getattr(ins, "sync_info", None)
                if si is not None and si.on_wait and len(si.on_wait) > max_waits:
                    waits = list(si.on_wait)
                    for w in waits[:-max_waits]:
                        ctr[0] += 1
                        nop = mybir.InstNoOp(
                            name=f"I-wsplit-{ctr[0]}", ins=[], outs=[],
                            sync_info=mybir.SyncInfo(on_wait=[w], on_update=[]),
                        )
                        nop.engine = ins.engine
                        out.append(nop)
                    ins.sync_info = mybir.SyncInfo(
                        on_wait=waits[-max_waits:],
                        on_update=list(si.on_update or []),
                    )
                out.append(ins)
            blk.instructions[:] = out
    return ctr[0]


_CACHE = {}
LAST_EXEC_NS = None
TRACE = False


def _install_ntff_shim():
    try:
        import antenv.axon_hooks  # noqa: F401
        return
    except ImportError:
        pass
    try:
        from trn_agent_boot.trn_boot import _ntff_profile_via_ctypes
        hook = _ntff_profile_via_ctypes("/opt/axon/libaxon_pjrt.so")
    except Exception:
        hook = None
    m1 = types.ModuleType("antenv")
    m2 = types.ModuleType("antenv.axon_hooks")
    m2.get_axon_ntff_profile_hook = lambda: hook
    m2.set_axon_ntff_profile_hook = lambda h: None
    m1.axon_hooks = m2
    sys.modules.setdefault("antenv", m1)
    sys.modules["antenv.axon_hooks"] = m2


def _prepare(inputs):
    inp = {k: np.asarray(v) for k, v in inputs.items()}
    ss = inp["span_starts"].astype(np.int64)
    sl = inp["span_lengths"].astype(np.int64)
    plan = _plan(ss, sl)
    T_cap, K_WIN, bases = plan["T_cap"], plan["K_WIN"], plan["bases"]
    KC = K_WIN // 128
    b3val = float(np.asarray(inp["score_b3"]).reshape(-1)[0])

    kcs = plan["kcs"]
    key = (T_cap, K_WIN, tuple(bases), tuple(kcs), b3val)
    if key not in _CACHE:
        _CACHE[key] = _build(T_cap, K_WIN, bases, kcs, b3val)
    nc = _CACHE[key]

    def bfc(x):
        return np.ascontiguousarray(np.asarray(x, dtype=np.float32)).astype(bf16)

    sw1 = inp["score_w1"].astype(np.float32)
    shared = {
        "aw1": bfc(inp["attn_w1"]),
        "aw2": bfc(inp["attn_w2"]),
        "aw3m": bfc(inp["attn_w3"].reshape(8, 128).T),
        "ab1m": np.ascontiguousarray(
            inp["attn_b1"].astype(np.float32).reshape(8, 128).T),
        "ab2m": np.ascontiguousarray(
            inp["attn_b2"].astype(np.float32).reshape(8, 128).T),
        "w1a": bfc(sw1[0:1024]),
        "w1b": bfc(sw1[1024:2048]),
        "w1c": bfc(sw1[2048:3072]),
        "w1d": bfc(sw1[3072:3092]),
        "wtT": bfc(inp["width_table"].T),
        "b1r": bfc(inp["score_b1"].reshape(1, HID)),
        "w2": bfc(inp["score_w2"]),
        "b2m": np.ascontiguousarray(
            inp["score_b2"].astype(np.float32).reshape(8, 128).T),
        "w3m": bfc(inp["score_w3"].reshape(8, 128).T),
        "iotaCh": np.ascontiguousarray(
            (np.arange(128, dtype=np.float32)[:, None]
             + 128.0 * np.arange(KC, dtype=np.float32)[None, :])
        ),
        "ident": np.eye(128, dtype=np.float32).astype(bf16),
    }

    states = inp["states"].astype(np.float32)
    embeds = inp["embeds"].astype(np.float32)
    in_maps = []
    for c in range(N_CORES):
        cb = int(plan["core_base"][c])
        stl = np.zeros((T_cap, D), np.float32)
        eml = np.zeros((T_cap, D), np.float32)
        hi = min(T, cb + T_cap)
        stl[: hi - cb] = states[cb:hi]
        eml[: hi - cb] = embeds[cb:hi]
        m = dict(shared)
        m["statesT"] = np.ascontiguousarray(stl.T).astype(bf16)
        m["embedsT"] = np.ascontiguousarray(eml.T).astype(bf16)
        d = plan["d"][c]
        dl = plan["dl"][c]
        ln = plan["ln"][c]
        m["dflat"] = d.reshape(1, C).astype(np.float16)
        m["deflat"] = dl.reshape(1, C).astype(np.float16)
        m["lenflat"] = ln.reshape(1, C).astype(np.float16)
        in_maps.append(m)

    return nc, in_maps, plan


def kernel(**inputs):
    global LAST_EXEC_NS
    from concourse.bass_utils import run_bass_kernel_spmd

    nc, in_maps, plan = _prepare(inputs)
    _install_ntff_shim()
    res = run_bass_kernel_spmd(nc, in_maps, list(range(N_CORES)), trace=TRACE)
    LAST_EXEC_NS = res.exec_time_ns

    out = np.empty(NSPAN, np.float32)
    for c in range(N_CORES):
        out[plan["order"][c * C : (c + 1) * C]] = np.asarray(
            res.results[c]["scores"]).reshape(-1)
    return out.reshape(NSPAN, 1)


# revision 12
# speedup vs baseline: 1.8388x; 1.0629x over previous
"""Trainium2 Bass kernel for nn_MentionScore.

Strategy: sort spans by start, shard 2048 consecutive sorted spans per core.
Each core only touches a ~1.1k-token window of states/embeds (host passes the
window pre-transposed, bf16). The ragged gather/softmax/weighted-sum becomes
dense matmuls against one-hot / banded matrices built on-device with
iota-compare tensor ops. Layer-1 of the span MLP is algebraically folded:
  h1 = relu(OH_s.T@P1 + OH_e.T@P2 + W.T@P3' + onehot(len).T@WB)
with P1=states@W1a, P2=states@W1b, P3'=exp(attn)*(embeds@W1c) per token and
WB = width_table@W1d + b1.

Softmax separability: exp(span_attns[n,l]) = exp(attns[start_n+l]) means the
softmax numerator matrix factors as diag(exp(attns)) @ Band where Band is the
0/1 in-range indicator. exp() is folded into P3 per token (P3'), the
denominator ssum = Band.T @ exp(attns) comes from an N=1 matmul riding the
Band stationary, and the weight band is W = Band * broadcast(1/ssum), with the
broadcast done by a K=1 PE outer product.
"""

import sys
import types

import numpy as np
import ml_dtypes

import concourse.bass as bass
import concourse.mybir as mybir
from concourse.ap import AP
from concourse.tile import TileContext
from concourse.vector_clock import ScopedClock

BF = mybir.dt.bfloat16
F16 = mybir.dt.float16
F32 = mybir.dt.float32
AT = mybir.AluOpType
AF = mybir.ActivationFunctionType
AX = mybir.AxisListType
bf16 = ml_dtypes.bfloat16

N_CORES = 8
T, NSPAN, D, HID, LMAX, WD = 8192, 16384, 1024, 1024, 10, 20
C = NSPAN // N_CORES          # spans per core
G = C // 128                  # 128-span groups per core


class PatchedTileContext(TileContext):
    """Workaround: walrus rejects the tail Drain when it carries >1 sem wait
    ("Too many sync wait commands"). Put each wait on its own NoOp instead."""

    def _drain_and_barrier(self, tick_clock, wait_clock):
        nc = self.nc
        drain_inst = nc.sync.drain()
        wait_clock.add_sem_waits(
            drain_inst.ins, ScopedClock({None: tick_clock.global_clock})
        )
        si = drain_inst.ins.sync_info
        if si is not None and si.on_wait is not None and len(si.on_wait) > 1:
            waits = list(si.on_wait)
            drain_inst.ins.sync_info = mybir.SyncInfo(
                on_wait=[waits[0]], on_update=list(si.on_update or [])
            )
            for w in waits[1:]:
                nop = nc.sync.nop()
                nop.ins.sync_info = mybir.SyncInfo(on_wait=[w], on_update=[])

        nc.all_engine_barrier()
        assert self.sems is not None
        popped = nc._tile_sem_poison_stack.pop()
        assert popped is self._sem_poison
        nc.clear_and_free_semaphores(list(self.sems.allocated().values()))
        nc.all_engine_barrier()


def _ceil128(x):
    return int(-(-int(x) // 128) * 128)


def _plan(span_starts, span_lengths):
    """Host-side sharding plan. Returns per-core data + static layout consts."""
    order = np.argsort(span_starts, kind="stable").astype(np.int64)
    ss = span_starts[order].reshape(N_CORES, C).astype(np.int64)
    sl = span_lengths[order].reshape(N_CORES, C).astype(np.int64)
    core_base = ss[:, 0].copy()
    sloc = ss - core_base[:, None]
    eloc = sloc + sl

    T_cap = _ceil128(int(eloc.max()) + 1)
    # unaligned, shared-across-cores group window bases + per-group k-tiles
    mn = sloc[:, ::128].min(axis=0)                       # [G]
    mx = eloc.reshape(N_CORES, G, 128).max(axis=2).max(axis=0)  # [G]
    need = mx - mn + 1
    kcs = np.maximum((need + 127) // 128, 1)
    T_pad = T_cap + 128
    bases = mn.copy()
    for _ in range(3):
        bases = np.minimum(mn, T_pad - kcs * 128)
        bad = (mx - bases + 1) > kcs * 128
        if not bad.any():
            break
        kcs[bad] += 1
    K_WIN = int(kcs.max()) * 128
    d = sloc - np.repeat(bases, 128)[None, :]
    assert d.min() >= 0 and ((d + sl).reshape(N_CORES, G, 128).max(axis=2)
                             <= kcs[None, :] * 128 - 1).all(), "window overflow"

    return {
        "order": order,
        "core_base": core_base,
        "sloc": sloc,
        "d": d.astype(np.float64),
        "dl": (d + sl).astype(np.float64),
        "ln": sl.astype(np.float64),
        "T_cap": T_cap,
        "K_WIN": int(K_WIN),
        "bases": [int(b) for b in bases],
        "kcs": [int(k) for k in kcs],
    }


NGROUPS = G
SPLIT_WAITS = True


def _build(T_cap, K_WIN, bases, kcs, b3val):
    """Build the single SPMD Bass program (static; shared by all 8 cores)."""
    KC = K_WIN // 128
    T_pad = T_cap + 128
    nc = bass.Bass()

    def par(name, shape, dt):
        return nc.declare_dram_parameter(name, list(shape), dt, isOutput=False)

    statesT_p = par("statesT", [D, T_cap], BF)
    embedsT_p = par("embedsT", [D, T_cap], BF)
    dflat_p = par("dflat", [1, C], F16)
    deflat_p = par("deflat", [1, C], F16)
    lenflat_p = par("lenflat", [1, C], F16)
    aw1_p = par("aw1", [D, HID], BF)
    aw2_p = par("aw2", [HID, HID], BF)
    aw3_p = par("aw3m", [128, 8], BF)
    ab1_p = par("ab1m", [128, 8], F32)
    ab2_p = par("ab2m", [128, 8], F32)
    w1a_p = par("w1a", [D, HID], BF)
    w1b_p = par("w1b", [D, HID], BF)
    w1c_p = par("w1c", [D, HID], BF)
    w1d_p = par("w1d", [WD, HID], BF)
    wtT_p = par("wtT", [WD, LMAX], BF)
    b1r_p = par("b1r", [1, HID], BF)
    w2_p = par("w2", [HID, HID], BF)
    b2_p = par("b2m", [128, 8], F32)
    w3_p = par("w3m", [128, 8], BF)
    iotaCh_p = par("iotaCh", [128, KC], F32)
    ident_p = par("ident", [128, 128], BF)
    scores_p = nc.declare_dram_parameter("scores", [1, C], F32, isOutput=True)

    with PatchedTileContext(nc) as tc:
        with (
            tc.tile_pool(name="pp", bufs=1) as pp,
            tc.tile_pool(name="wst", bufs=2) as wst,
            tc.tile_pool(name="gp", bufs=2) as gp,
            tc.tile_pool(name="ps", bufs=2, space="PSUM") as ps,
            tc.tile_pool(name="dp", bufs=1, space="DRAM") as dp,
        ):
            dma = nc.sync.dma_start
            dmas = nc.scalar.dma_start

            # ---------- weights needed by block 0 go FIRST (PE warmup) -----
            def wload(param, tag_prefix):
                tiles = []
                for k in range(8):
                    t = pp.tile([128, HID], BF, name=f"{tag_prefix}{k}",
                                tag=f"{tag_prefix}{k}")
                    dma(out=t[:], in_=param[k * 128 : (k + 1) * 128, :])
                    tiles.append(t)
                return tiles

            ab1_t = pp.tile([128, 8], F32, name="ab1", tag="ab1")
            dma(out=ab1_t[:], in_=ab1_p[:])
            ab2_t = pp.tile([128, 8], F32, name="ab2", tag="ab2")
            dma(out=ab2_t[:], in_=ab2_p[:])
            aw3_t = pp.tile([128, 8], BF, name="aw3", tag="aw3")
            dma(out=aw3_t[:], in_=aw3_p[:])
            aw2_t = w1a_t = w1b_t = w1c_t = None

            b2_t = pp.tile([128, 8], F32, name="b2", tag="b2")
            dma(out=b2_t[:], in_=b2_p[:])
            w3_t = pp.tile([128, 8], BF, name="w3", tag="w3")
            dma(out=w3_t[:], in_=w3_p[:])
            b1r_t = pp.tile([1, HID], BF, name="b1r", tag="b1r")
            dmas(out=b1r_t[:], in_=b1r_p[:])
            w1d_t = pp.tile([WD, HID], BF, name="w1d", tag="w1d")
            dmas(out=w1d_t[:], in_=w1d_p[:])
            wtT_t = pp.tile([WD, 16], BF, name="wtT", tag="wtT")
            nc.vector.memset(wtT_t[:], 0.0)
            dmas(out=wtT_t[:, :LMAX], in_=wtT_p[:])
            ident_t = pp.tile([128, 128], BF, name="ident", tag="ident")
            dmas(out=ident_t[:], in_=ident_p[:])
            iotaCh_t = pp.tile([128, KC], F32, name="iotaCh", tag="iotaCh")
            dmas(out=iotaCh_t[:], in_=iotaCh_p[:])

            ones11 = pp.tile([1, 1], BF, name="ones11", tag="ones11")
            nc.vector.memset(ones11[:], 1.0)
            ones128 = pp.tile([1, 128], BF, name="ones128", tag="ones128")
            nc.vector.memset(ones128[:], 1.0)
            ones16_t = pp.tile([1, 16], BF, name="ones16", tag="ones16")
            nc.vector.memset(ones16_t[:], 1.0)

            ea_dram = dp.tile(
                [T_pad + 16], BF, name="ea_dram", tag="ea_dram")

            if NGROUPS < G:  # debug builds: ensure output is written
                zsc = pp.tile([1, C], F32, name="zsc", tag="zsc")
                nc.vector.memset(zsc[:], 0.0)
                dma(out=scores_p[:], in_=zsc[:])

            # ---------- P targets in DRAM [T_pad, HID] ----------
            P1d = dp.tile([T_pad, HID], BF, name="P1d", tag="P1d")
            P2d = dp.tile([T_pad, HID], BF, name="P2d", tag="P2d")
            P3d = dp.tile([T_pad, HID], BF, name="P3d", tag="P3d")
            Pd = (P1d, P2d, P3d)
            zrow = pp.tile([128, 512], BF, name="zrow", tag="zrow")
            nc.vector.memset(zrow[:], 0.0)

            # ---------- blocked token pipeline: attn MLP + P projections --
            ea_t = pp.tile([1, T_cap], BF, name="ea_t", tag="ea_t")
            _aw1_stage = []
            nblocks = [(n0, min(512, T_cap - n0)) for n0 in range(0, T_cap, 512)]

            for n0, nw in nblocks:
                sTw = []
                eTw = []
                for k in range(8):
                    if n0 == 0:
                        # interleave weight/data DMAs so the first L1 chain's
                        # operands arrive earliest
                        t = pp.tile([128, HID], BF, name=f"wA{k}",
                                    tag=f"wA{k}")
                        dma(out=t[:], in_=aw1_p[k * 128 : (k + 1) * 128, :])
                        _aw1_stage.append(t)
                    ts_ = wst.tile([128, 512], BF, name=f"sTw{k}", tag=f"sTw{k}")
                    dma(out=ts_[:, :nw],
                        in_=statesT_p[k * 128 : (k + 1) * 128, n0 : n0 + nw])
                    sTw.append(ts_)
                aw1_t = _aw1_stage
                for k in range(8):
                    te_ = wst.tile([128, 512], BF, name=f"eTw{k}", tag=f"eTw{k}",
                                   bufs=1)
                    dma(out=te_[:, :nw],
                        in_=embedsT_p[k * 128 : (k + 1) * 128, n0 : n0 + nw])
                    eTw.append(te_)
                if aw2_t is None:
                    aw2_t = wload(aw2_p, "wB")
                h1a = [wst.tile([128, 512], BF, name=f"h1a{h}", tag=f"h1a{h}",
                                bufs=1)
                       for h in range(8)]
                h2a = [wst.tile([128, 512], BF, name=f"h2a{h}", tag=f"h2a{h}",
                                bufs=1)
                       for h in range(8)]
                for hc in range(8):
                    pt = ps.tile([128, 512], F32, name="big", tag="big", bufs=2)
                    for k in range(8):
                        nc.tensor.matmul(
                            pt[:, :nw],
                            aw1_t[k][:, hc * 128 : (hc + 1) * 128],
                            sTw[k][:, :nw], start=(k == 0), stop=(k == 7))
                    nc.scalar.activation(
                        h1a[hc][:, :nw], pt[:, :nw], AF.Relu,
                        bias=ab1_t[:, hc : hc + 1])
                for hc in range(8):
                    pt = ps.tile([128, 512], F32, name="big", tag="big", bufs=2)
                    for k in range(8):
                        nc.tensor.matmul(
                            pt[:, :nw],
                            aw2_t[k][:, hc * 128 : (hc + 1) * 128],
                            h1a[k][:, :nw], start=(k == 0), stop=(k == 7))
                    nc.scalar.activation(
                        h2a[hc][:, :nw], pt[:, :nw], AF.Relu,
                        bias=ab2_t[:, hc : hc + 1])
                pt = ps.tile([1, 512], F32, name="big", tag="big", bufs=2)
                for k in range(8):
                    nc.tensor.matmul(
                        pt[:, :nw], aw3_t[:, k : k + 1], h2a[k][:, :nw],
                        start=(k == 0), stop=(k == 7))
                nc.scalar.activation(ea_t[:, n0 : n0 + nw], pt[:, :nw], AF.Exp)
                dmas(out=ea_dram[n0 : n0 + nw], in_=ea_t[0:1, n0 : n0 + nw])
                # per-128-token projections into DRAM
                if w1a_t is None:
                    w1a_t = wload(w1a_p, "wWA")
                    w1b_t = wload(w1b_p, "wWB")
                    w1c_t = wload(w1c_p, "wWC")

                def proj(pi, wt_, srcs, j, scale_ap=None):
                    js = slice(j * 128, (j + 1) * 128)
                    for h0 in (0, 512):
                        pt = ps.tile([128, 512], F32, name="big",
                                     tag="big", bufs=2)
                        for k in range(8):
                            nc.tensor.matmul(
                                pt[:], srcs[k][:, js],
                                wt_[k][:, h0 : h0 + 512],
                                start=(k == 0), stop=(k == 7))
                        stg = wst.tile([128, 512], BF, name=f"pstg{pi}",
                                       tag=f"pstg{pi}", bufs=2)
                        if scale_ap is None:
                            nc.scalar.copy(stg[:], pt[:])
                        else:
                            nc.scalar.mul(stg[:], pt[:], scale_ap)
                        dmas(out=Pd[pi][n0 + j * 128 : n0 + (j + 1) * 128,
                                        h0 : h0 + 512], in_=stg[:])

                nj = nw // 128
                for j in range(nj):
                    proj(0, w1a_t, sTw, j)
                    proj(1, w1b_t, sTw, j)
                # exp(attn) columns for this block: [1,128] rows -> [128,1]
                eac_sb = wst.tile([128, 4], F32, name="eacs", tag="eacs",
                                  bufs=2)
                eac_ps = ps.tile([128, 128], F32, name="eac", tag="rbc",
                                 bufs=1)
                for j in range(nj):
                    nc.tensor.matmul(
                        eac_ps[:, j : j + 1],
                        ea_t[0:1, n0 + j * 128 : n0 + (j + 1) * 128],
                        ones11[:], start=True, stop=True)
                nc.vector.tensor_copy(out=eac_sb[:, :nj], in_=eac_ps[:, :nj])
                for j in range(nj):
                    proj(2, w1c_t, eTw, j, scale_ap=eac_sb[:, j : j + 1])

            # pad P + ea beyond T_cap
            for pd in Pd:
                dma(out=pd[T_cap:, 0:512], in_=zrow[:])
                dma(out=pd[T_cap:, 512:1024], in_=zrow[:])
            zpad_t = pp.tile([1, 144], BF, name="zpad", tag="zpad")
            nc.vector.memset(zpad_t[:], 0.0)
            dma(out=ea_dram[T_cap:], in_=zpad_t[0:1, :])

            # span-index broadcasts (fp16, exact for ints < 2048)
            dbc = pp.tile([128, C], F16, name="dbc", tag="dbc")
            dma(out=dbc[:], in_=dflat_p[:].partition_broadcast(128))
            debc = pp.tile([128, C], F16, name="debc", tag="debc")
            dma(out=debc[:], in_=deflat_p[:].partition_broadcast(128))
            lnbc = pp.tile([16, C], F16, name="lnbc", tag="lnbc")
            dma(out=lnbc[:], in_=lenflat_p[:].partition_broadcast(16))

            # ---------- WB = width_table @ W1d + b1  → [16, HID] bf16 ------
            WB_t = pp.tile([16, HID], BF, name="WB", tag="WB")
            for h0 in range(0, HID, 512):
                pt = ps.tile([16, 512], F32, name="big", tag="big", bufs=2)
                nc.tensor.matmul(pt[:], wtT_t[:], w1d_t[:, h0 : h0 + 512],
                                 start=True, stop=False)
                nc.tensor.matmul(pt[:], ones16_t[:], b1r_t[:, h0 : h0 + 512],
                                 start=False, stop=True)
                nc.vector.tensor_copy(out=WB_t[:, h0 : h0 + 512], in_=pt[:])

            ea_h = ea_dram[:]            # AP to get the dram tensor handle

            # ---------- pre-pass: ssum + 1/ssum per group ----------
            rT_all = pp.tile([1, C], BF, name="rT_all", tag="rT_all")
            ea_all = pp.tile([128, G * KC], BF, name="ea_all", tag="ea_all")

            def build_band(g, kk):
                gs = slice(g * 128, (g + 1) * 128)
                tle = gp.tile([128, 128], F16, name="tle", tag="tle", bufs=1)
                nc.vector.tensor_scalar(
                    out=tle[:], in0=dbc[:, gs],
                    scalar1=iotaCh_t[:, kk : kk + 1], scalar2=None,
                    op0=AT.is_le)
                tge = gp.tile([128, 128], F16, name="tge", tag="tge", bufs=1)
                nc.vector.tensor_scalar(
                    out=tge[:], in0=debc[:, gs],
                    scalar1=iotaCh_t[:, kk : kk + 1], scalar2=None,
                    op0=AT.is_ge)
                band = gp.tile([128, 128], BF, name="band", tag="band", bufs=1)
                nc.vector.tensor_tensor(out=band[:], in0=tle[:], in1=tge[:],
                                        op=AT.mult)
                return band

            ssum_ps = ps.tile([128, 128], F32, name="ssum", tag="rbc",
                              bufs=1)
            for g in range(NGROUPS):
                KCg = kcs[g]
                dma(out=ea_all[:, g * KC : g * KC + KCg],
                     in_=AP(tensor=ea_h.tensor, offset=bases[g],
                            ap=[[1, 128], [128, KCg]]))
                for kk in range(KCg):
                    band = build_band(g, kk)
                    nc.tensor.matmul(
                        ssum_ps[:, g : g + 1], band[:],
                        ea_all[:, g * KC + kk : g * KC + kk + 1],
                        start=(kk == 0), stop=(kk == KCg - 1))
            rinv32 = gp.tile([128, G], F32, name="rinv32", tag="rinv32")
            nc.vector.reciprocal(rinv32[:], ssum_ps[:, :G])
            rinvbf = gp.tile([128, G], BF, name="rinvbf", tag="rinvbf")
            nc.vector.tensor_copy(out=rinvbf[:], in_=rinv32[:])
            rT8 = None
            for g in range(NGROUPS):
                if g % 8 == 0:
                    rT8 = ps.tile([128, 8, 128], BF, name="tr", tag="tr",
                                  bufs=1)
                nc.tensor.transpose(rT8[0:1, g % 8, :],
                                    rinvbf[:, g : g + 1], ident_t[:])
                if g % 8 == 7:
                    nc.vector.tensor_copy(
                        out=rT_all[:, (g - 7) * 128 : (g + 1) * 128],
                        in_=rT8[0:1, :, :])

            # ---------- span groups ----------
            w2_t = wload(w2_p, "wA")     # reuse token-weight slots for L2

            def prep_group(g):
                """One-hots + softmax weight band for group g (runs one
                group ahead of the consuming h1 chain)."""
                KCg = kcs[g]
                gs = slice(g * 128, (g + 1) * 128)
                rbc_ps = ps.tile([128, 128], F32, name="rbc", tag="rbc",
                                 bufs=1)
                nc.tensor.matmul(rbc_ps[:], ones128[:], rT_all[0:1, gs],
                                 start=True, stop=True)
                ohs, ohe, wt = [], [], []
                for kk in range(KCg):
                    o1 = gp.tile([128, 128], BF, name=f"ohs{kk}",
                                 tag=f"ohs{kk}", bufs=2)
                    nc.vector.tensor_scalar(
                        out=o1[:], in0=dbc[:, gs],
                        scalar1=iotaCh_t[:, kk : kk + 1], scalar2=None,
                        op0=AT.is_equal)
                    ohs.append(o1)
                    o2 = gp.tile([128, 128], BF, name=f"ohe{kk}",
                                 tag=f"ohe{kk}", bufs=2)
                    nc.vector.tensor_scalar(
                        out=o2[:], in0=debc[:, gs],
                        scalar1=iotaCh_t[:, kk : kk + 1], scalar2=None,
                        op0=AT.is_equal)
                    ohe.append(o2)
                    band = build_band(g, kk)
                    w_ = gp.tile([128, 128], BF, name=f"wt{kk}",
                                 tag=f"wt{kk}", bufs=2)
                    nc.vector.tensor_tensor(out=w_[:], in0=band[:],
                                            in1=rbc_ps[:], op=AT.mult)
                    wt.append(w_)
                ohl = gp.tile([16, 128], BF, name="ohl", tag="ohl", bufs=2)
                nc.vector.tensor_scalar(
                    out=ohl[:], in0=lnbc[:, gs],
                    scalar1=iotaCh_t[:16, 0:1], scalar2=None,
                    op0=AT.is_equal)
                return (ohs, ohe, wt, ohl)

            def pw_dmas(g):
                KCg = kcs[g]
                pw = []
                for pi in range(3):
                    tiles = []
                    for kk in range(KCg):
                        pt_ = wst.tile([128, HID], BF, name=f"pw{pi}_{kk}",
                                       tag=f"pw{pi}_{kk}", bufs=2)
                        r0 = bases[g] + kk * 128
                        dma(out=pt_[:], in_=Pd[pi][r0 : r0 + 128, :])
                        tiles.append(pt_)
                    pw.append(tiles)
                return pw

            h1b = h2b = None
            prep = prep_group(0)
            pwcur = pw_dmas(0)
            for g in range(NGROUPS):
                KCg = kcs[g]
                if g % 4 == 0:
                    h1b = gp.tile([128, 8, 512], BF, name="h1b", tag="h1b",
                                  bufs=1)
                gcol = (g % 4) * 128

                ohs, ohe, wt, ohl = prep
                pw = pwcur
                if g + 1 < NGROUPS:
                    pwcur = pw_dmas(g + 1)

                # h1[n, h]: one-hot stationary, P moving (dense M=512 MMs)
                hp = ps.tile([128, 1024], F32, name="h1f", tag="h1f",
                             bufs=2)
                steps = []
                for kk in range(KCg):
                    steps.append((ohs[kk][:], pw[0][kk]))
                    steps.append((ohe[kk][:], pw[1][kk]))
                steps.append((ohl[:], None))
                for kk in range(KCg):
                    steps.append((wt[kk][:], pw[2][kk]))
                ns = len(steps)
                for h0 in (0, 512):
                    for i, (lhsT, rhs_t) in enumerate(steps):
                        rhs = (WB_t[:, h0 : h0 + 512] if rhs_t is None
                               else rhs_t[:, h0 : h0 + 512])
                        nc.tensor.matmul(
                            hp[:, h0 : h0 + 512], lhsT, rhs,
                            start=(i == 0), stop=(i == ns - 1))
                h1s = gp.tile([128, 1024], BF, name="h1s", tag="h1s",
                              bufs=2)
                nc.vector.tensor_scalar(
                    out=h1s[:], in0=hp[:], scalar1=0.0, scalar2=None,
                    op0=AT.max)
                tr = ps.tile([128, 8, 128], BF, name="tr", tag="tr",
                             bufs=1)
                for k in range(8):
                    nc.tensor.transpose(
                        tr[:, k, :], h1s[:, k * 128 : (k + 1) * 128],
                        ident_t[:])
                nc.vector.tensor_copy(
                    out=h1b[:, :, gcol : gcol + 128], in_=tr[:])
                if g + 1 < NGROUPS:
                    prep = prep_group(g + 1)

                # every 4 groups: span-MLP L2+L3 on the 512-col block
                if g % 4 == 3:
                    b0 = (g // 4) * 512
                    h2b = [gp.tile([128, 512], BF, name=f"h2b{k}",
                                   tag=f"h2b{k}", bufs=1)
                           for k in range(8)]
                    for h2c in range(8):
                        pt = ps.tile([128, 512], F32, name="big", tag="big",
                                     bufs=2)
                        for k in range(8):
                            nc.tensor.matmul(
                                pt[:], w2_t[k][:, h2c * 128 : (h2c + 1) * 128],
                                h1b[:, k, :], start=(k == 0), stop=(k == 7))
                        nc.vector.tensor_scalar(
                            out=h2b[h2c][:], in0=pt[:],
                            scalar1=b2_t[:, h2c : h2c + 1], scalar2=0.0,
                            op0=AT.add, op1=AT.max)
                    pt = ps.tile([1, 512], F32, name="big", tag="big", bufs=2)
                    for k in range(8):
                        nc.tensor.matmul(pt[:], w3_t[:, k : k + 1], h2b[k][:],
                                         start=(k == 0), stop=(k == 7))
                    ob = gp.tile([1, 512], F32, name="ob", tag="ob")
                    nc.vector.tensor_scalar(out=ob[:], in0=pt[:],
                                            scalar1=float(b3val), scalar2=None,
                                            op0=AT.add)
                    dma(out=scores_p[:, b0 : b0 + 512], in_=ob[:])

    if SPLIT_WAITS:
        _split_waits(nc)
    return nc



def _split_waits(nc, max_waits=1):
    """This walrus build rejects instructions carrying >max_waits sem waits
    ("Too many sync wait commands"). Hoist excess waits onto same-engine
    NoOps placed immediately before the instruction — identical semantics
    (engine queues are in-order)."""
    ctr = [0]
    for f in nc.m.functions:
        for blk in f.blocks:
            out = []
            for ins in blk.instructions:
                si = getattr(ins, "sync_info", None)
                if si is not None and si.on_wait and len(si.on_wait) > max_waits:
                    waits = list(si.on_wait)
                    for w in waits[:-max_waits]:
                        ctr[0] += 1
                        nop = mybir.InstNoOp(
                            name=f"I-wsplit-{ctr[0]}", ins=[], outs=[],
                            sync_info=mybir.SyncInfo(on_wait=[w], on_update=[]),
                        )
                        nop.engine = ins.engine
                        out.append(nop)
                    ins.sync_info = mybir.SyncInfo(
                        on_wait=waits[-max_waits:],
                        on_update=list(si.on_update or []),
                    )
                out.append(ins)
            blk.instructions[:] = out
    return ctr[0]


_CACHE = {}
LAST_EXEC_NS = None
TRACE = False


def _install_ntff_shim():
    try:
        import antenv.axon_hooks  # noqa: F401
        return
    except ImportError:
        pass
    try:
        from trn_agent_boot.trn_boot import _ntff_profile_via_ctypes
        hook = _ntff_profile_via_ctypes("/opt/axon/libaxon_pjrt.so")
    except Exception:
        hook = None
    m1 = types.ModuleType("antenv")
    m2 = types.ModuleType("antenv.axon_hooks")
    m2.get_axon_ntff_profile_hook = lambda: hook
    m2.set_axon_ntff_profile_hook = lambda h: None
    m1.axon_hooks = m2
    sys.modules.setdefault("antenv", m1)
    sys.modules["antenv.axon_hooks"] = m2


def _prepare(inputs):
    inp = {k: np.asarray(v) for k, v in inputs.items()}
    ss = inp["span_starts"].astype(np.int64)
    sl = inp["span_lengths"].astype(np.int64)
    plan = _plan(ss, sl)
    T_cap, K_WIN, bases = plan["T_cap"], plan["K_WIN"], plan["bases"]
    KC = K_WIN // 128
    b3val = float(np.asarray(inp["score_b3"]).reshape(-1)[0])

    kcs = plan["kcs"]
    key = (T_cap, K_WIN, tuple(bases), tuple(kcs), b3val)
    if key not in _CACHE:
        _CACHE[key] = _build(T_cap, K_WIN, bases, kcs, b3val)
    nc = _CACHE[key]

    def bfc(x):
        return np.ascontiguousarray(np.asarray(x, dtype=np.float32)).astype(bf16)

    sw1 = inp["score_w1"].astype(np.float32)
    shared = {
        "aw1": bfc(inp["attn_w1"]),
        "aw2": bfc(inp["attn_w2"]),
        "aw3m": bfc(inp["attn_w3"].reshape(8, 128).T),
        "ab1m": np.ascontiguousarray(
            inp["attn_b1"].astype(np.float32).reshape(8, 128).T),
        "ab2m": np.ascontiguousarray(
            inp["attn_b2"].astype(np.float32).reshape(8, 128).T),
        "w1a": bfc(sw1[0:1024]),
        "w1b": bfc(sw1[1024:2048]),
        "w1c": bfc(sw1[2048:3072]),
        "w1d": bfc(sw1[3072:3092]),
        "wtT": bfc(inp["width_table"].T),
        "b1r": bfc(inp["score_b1"].reshape(1, HID)),
        "w2": bfc(inp["score_w2"]),
        "b2m": np.ascontiguousarray(
            inp["score_b2"].astype(np.float32).reshape(8, 128).T),
        "w3m": bfc(inp["score_w3"].reshape(8, 128).T),
        "iotaCh": np.ascontiguousarray(
            (np.arange(128, dtype=np.float32)[:, None]
             + 128.0 * np.arange(KC, dtype=np.float32)[None, :])
        ),
        "ident": np.eye(128, dtype=np.float32).astype(bf16),
    }

    states = inp["states"].astype(np.float32)
    embeds = inp["embeds"].astype(np.float32)
    in_maps = []
    for c in range(N_CORES):
        cb = int(plan["core_base"][c])
        stl = np.zeros((T_cap, D), np.float32)
        eml = np.zeros((T_cap, D), np.float32)
        hi = min(T, cb + T_cap)
        stl[: hi - cb] = states[cb:hi]
        eml[: hi - cb] = embeds[cb:hi]
        m = dict(shared)
        m["statesT"] = np.ascontiguousarray(stl.T).astype(bf16)
        m["embedsT"] = np.ascontiguousarray(eml.T).astype(bf16)
        d = plan["d"][c]
        dl = plan["dl"][c]
        ln = plan["ln"][c]
        m["dflat"] = d.reshape(1, C).astype(np.float16)
        m["deflat"] = dl.reshape(1, C).astype(np.float16)
        m["lenflat"] = ln.reshape(1, C).astype(np.float16)
        in_maps.append(m)

    return nc, in_maps, plan


def kernel(**inputs):
    global LAST_EXEC_NS
    from concourse.bass_utils import run_bass_kernel_spmd

    nc, in_maps, plan = _prepare(inputs)
    _install_ntff_shim()
    res = run_bass_kernel_spmd(nc, in_maps, list(range(N_CORES)), trace=TRACE)
    LAST_EXEC_NS = res.exec_time_ns

    out = np.empty(NSPAN, np.float32)
    for c in range(N_CORES):
        out[plan["order"][c * C : (c + 1) * C]] = np.asarray(
            res.results[c]["scores"]).reshape(-1)
    return out.reshape(NSPAN, 1)


# revision 13
# speedup vs baseline: 1.8773x; 1.0210x over previous
"""Trainium2 Bass kernel for nn_MentionScore.

Strategy: sort spans by start, shard 2048 consecutive sorted spans per core.
Each core only touches a ~1.1k-token window of states/embeds (host passes the
window pre-transposed, bf16). The ragged gather/softmax/weighted-sum becomes
dense matmuls against one-hot / banded matrices built on-device with
iota-compare tensor ops. Layer-1 of the span MLP is algebraically folded:
  h1 = relu(OH_s.T@P1 + OH_e.T@P2 + W.T@P3' + onehot(len).T@WB)
with P1=states@W1a, P2=states@W1b, P3'=exp(attn)*(embeds@W1c) per token and
WB = width_table@W1d + b1.

Softmax separability: exp(span_attns[n,l]) = exp(attns[start_n+l]) means the
softmax numerator matrix factors as diag(exp(attns)) @ Band where Band is the
0/1 in-range indicator. exp() is folded into P3 per token (P3'), the
denominator ssum = Band.T @ exp(attns) comes from an N=1 matmul riding the
Band stationary, and the weight band is W = Band * broadcast(1/ssum), with the
broadcast done by a K=1 PE outer product.
"""

import sys
import types

import numpy as np
import ml_dtypes

import concourse.bass as bass
import concourse.mybir as mybir
from concourse.ap import AP
from concourse.tile import TileContext
from concourse.vector_clock import ScopedClock

BF = mybir.dt.bfloat16
F16 = mybir.dt.float16
F32 = mybir.dt.float32
AT = mybir.AluOpType
AF = mybir.ActivationFunctionType
AX = mybir.AxisListType
bf16 = ml_dtypes.bfloat16

N_CORES = 8
T, NSPAN, D, HID, LMAX, WD = 8192, 16384, 1024, 1024, 10, 20
C = NSPAN // N_CORES          # spans per core
G = C // 128                  # 128-span groups per core


class PatchedTileContext(TileContext):
    """Workaround: walrus rejects the tail Drain when it carries >1 sem wait
    ("Too many sync wait commands"). Put each wait on its own NoOp instead."""

    def _drain_and_barrier(self, tick_clock, wait_clock):
        nc = self.nc
        drain_inst = nc.sync.drain()
        wait_clock.add_sem_waits(
            drain_inst.ins, ScopedClock({None: tick_clock.global_clock})
        )
        si = drain_inst.ins.sync_info
        if si is not None and si.on_wait is not None and len(si.on_wait) > 1:
            waits = list(si.on_wait)
            drain_inst.ins.sync_info = mybir.SyncInfo(
                on_wait=[waits[0]], on_update=list(si.on_update or [])
            )
            for w in waits[1:]:
                nop = nc.sync.nop()
                nop.ins.sync_info = mybir.SyncInfo(on_wait=[w], on_update=[])

        nc.all_engine_barrier()
        assert self.sems is not None
        popped = nc._tile_sem_poison_stack.pop()
        assert popped is self._sem_poison
        nc.clear_and_free_semaphores(list(self.sems.allocated().values()))
        nc.all_engine_barrier()


def _ceil128(x):
    return int(-(-int(x) // 128) * 128)


def _plan(span_starts, span_lengths):
    """Host-side sharding plan. Returns per-core data + static layout consts."""
    order = np.argsort(span_starts, kind="stable").astype(np.int64)
    ss = span_starts[order].reshape(N_CORES, C).astype(np.int64)
    sl = span_lengths[order].reshape(N_CORES, C).astype(np.int64)
    core_base = ss[:, 0].copy()
    sloc = ss - core_base[:, None]
    eloc = sloc + sl

    T_cap = _ceil128(int(eloc.max()) + 1)
    # unaligned, shared-across-cores group window bases + per-group k-tiles
    mn = sloc[:, ::128].min(axis=0)                       # [G]
    mx = eloc.reshape(N_CORES, G, 128).max(axis=2).max(axis=0)  # [G]
    need = mx - mn + 1
    kcs = np.maximum((need + 127) // 128, 1)
    T_pad = T_cap + 128
    bases = mn.copy()
    for _ in range(3):
        bases = np.minimum(mn, T_pad - kcs * 128)
        bad = (mx - bases + 1) > kcs * 128
        if not bad.any():
            break
        kcs[bad] += 1
    K_WIN = int(kcs.max()) * 128
    d = sloc - np.repeat(bases, 128)[None, :]
    assert d.min() >= 0 and ((d + sl).reshape(N_CORES, G, 128).max(axis=2)
                             <= kcs[None, :] * 128 - 1).all(), "window overflow"

    return {
        "order": order,
        "core_base": core_base,
        "sloc": sloc,
        "d": d.astype(np.float64),
        "dl": (d + sl).astype(np.float64),
        "ln": sl.astype(np.float64),
        "T_cap": T_cap,
        "K_WIN": int(K_WIN),
        "bases": [int(b) for b in bases],
        "kcs": [int(k) for k in kcs],
    }


NGROUPS = G
SPLIT_WAITS = True


def _build(T_cap, K_WIN, bases, kcs, b3val):
    """Build the single SPMD Bass program (static; shared by all 8 cores)."""
    KC = K_WIN // 128
    T_pad = T_cap + 128
    nc = bass.Bass()

    def par(name, shape, dt):
        return nc.declare_dram_parameter(name, list(shape), dt, isOutput=False)

    statesT_p = par("statesT", [D, T_cap], BF)
    embedsT_p = par("embedsT", [D, T_cap], BF)
    dflat_p = par("dflat", [1, C], F16)
    deflat_p = par("deflat", [1, C], F16)
    lenflat_p = par("lenflat", [1, C], F16)
    aw1_p = par("aw1", [D, HID], BF)
    aw2_p = par("aw2", [HID, HID], BF)
    aw3_p = par("aw3m", [128, 8], BF)
    ab1_p = par("ab1m", [128, 8], F32)
    ab2_p = par("ab2m", [128, 8], F32)
    w1a_p = par("w1a", [D, HID], BF)
    w1b_p = par("w1b", [D, HID], BF)
    w1c_p = par("w1c", [D, HID], BF)
    w1d_p = par("w1d", [WD, HID], BF)
    wtT_p = par("wtT", [WD, LMAX], BF)
    b1r_p = par("b1r", [1, HID], BF)
    w2_p = par("w2", [HID, HID], BF)
    b2_p = par("b2m", [128, 8], F32)
    w3_p = par("w3m", [128, 8], BF)
    iotaCh_p = par("iotaCh", [128, KC], F32)
    ident_p = par("ident", [128, 128], BF)
    scores_p = nc.declare_dram_parameter("scores", [1, C], F32, isOutput=True)

    with PatchedTileContext(nc) as tc:
        with (
            tc.tile_pool(name="pp", bufs=1) as pp,
            tc.tile_pool(name="wst", bufs=2) as wst,
            tc.tile_pool(name="gp", bufs=2) as gp,
            tc.tile_pool(name="ps", bufs=2, space="PSUM") as ps,
            tc.tile_pool(name="dp", bufs=1, space="DRAM") as dp,
        ):
            dma = nc.sync.dma_start
            dmas = nc.scalar.dma_start

            # ---------- weights needed by block 0 go FIRST (PE warmup) -----
            def wload(param, tag_prefix):
                tiles = []
                for k in range(8):
                    t = pp.tile([128, HID], BF, name=f"{tag_prefix}{k}",
                                tag=f"{tag_prefix}{k}")
                    dma(out=t[:], in_=param[k * 128 : (k + 1) * 128, :])
                    tiles.append(t)
                return tiles

            ab1_t = pp.tile([128, 8], F32, name="ab1", tag="ab1")
            dma(out=ab1_t[:], in_=ab1_p[:])
            ab2_t = pp.tile([128, 8], F32, name="ab2", tag="ab2")
            dma(out=ab2_t[:], in_=ab2_p[:])
            aw3_t = pp.tile([128, 8], BF, name="aw3", tag="aw3")
            dma(out=aw3_t[:], in_=aw3_p[:])
            aw2_t = w1a_t = w1b_t = w1c_t = None

            b2_t = pp.tile([128, 8], F32, name="b2", tag="b2")
            dma(out=b2_t[:], in_=b2_p[:])
            w3_t = pp.tile([128, 8], BF, name="w3", tag="w3")
            dma(out=w3_t[:], in_=w3_p[:])
            b1r_t = pp.tile([1, HID], BF, name="b1r", tag="b1r")
            dmas(out=b1r_t[:], in_=b1r_p[:])
            w1d_t = pp.tile([WD, HID], BF, name="w1d", tag="w1d")
            dmas(out=w1d_t[:], in_=w1d_p[:])
            wtT_t = pp.tile([WD, 16], BF, name="wtT", tag="wtT")
            nc.vector.memset(wtT_t[:], 0.0)
            dmas(out=wtT_t[:, :LMAX], in_=wtT_p[:])
            ident_t = pp.tile([128, 128], BF, name="ident", tag="ident")
            dmas(out=ident_t[:], in_=ident_p[:])
            iotaCh_t = pp.tile([128, KC], F32, name="iotaCh", tag="iotaCh")
            dmas(out=iotaCh_t[:], in_=iotaCh_p[:])

            ones11 = pp.tile([1, 1], BF, name="ones11", tag="ones11")
            nc.vector.memset(ones11[:], 1.0)
            ones128 = pp.tile([1, 128], BF, name="ones128", tag="ones128")
            nc.vector.memset(ones128[:], 1.0)
            ones16_t = pp.tile([1, 16], BF, name="ones16", tag="ones16")
            nc.vector.memset(ones16_t[:], 1.0)

            ea_dram = dp.tile(
                [T_pad + 16], BF, name="ea_dram", tag="ea_dram")

            # span-index broadcasts (fp16, exact for ints < 2048)
            dbc = pp.tile([128, C], F16, name="dbc", tag="dbc")
            dmas(out=dbc[:], in_=dflat_p[:].partition_broadcast(128))
            debc = pp.tile([128, C], F16, name="debc", tag="debc")
            dmas(out=debc[:], in_=deflat_p[:].partition_broadcast(128))
            lnbc = pp.tile([16, C], F16, name="lnbc", tag="lnbc")
            dmas(out=lnbc[:], in_=lenflat_p[:].partition_broadcast(16))

            if NGROUPS < G:  # debug builds: ensure output is written
                zsc = pp.tile([1, C], F32, name="zsc", tag="zsc")
                nc.vector.memset(zsc[:], 0.0)
                dma(out=scores_p[:], in_=zsc[:])

            # ---------- P targets in DRAM [T_pad, HID] ----------
            P1d = dp.tile([T_pad, HID], BF, name="P1d", tag="P1d")
            P2d = dp.tile([T_pad, HID], BF, name="P2d", tag="P2d")
            P3d = dp.tile([T_pad, HID], BF, name="P3d", tag="P3d")
            Pd = (P1d, P2d, P3d)
            zrow = pp.tile([128, 512], BF, name="zrow", tag="zrow")
            nc.vector.memset(zrow[:], 0.0)

            # ---------- blocked token pipeline: attn MLP + P projections --
            ea_t = pp.tile([1, T_cap], BF, name="ea_t", tag="ea_t")
            ea_h = ea_dram[:]            # AP handle for column gathers
            ea_all = pp.tile([128, G * KC], BF, name="ea_all", tag="ea_all")
            ssum_sb = gp.tile([128, G], F32, name="ssum_sb", tag="ssum_sb")

            def build_band(g, kk):
                gs = slice(g * 128, (g + 1) * 128)
                tle = gp.tile([128, 128], F16, name="tle", tag="tle", bufs=1)
                nc.vector.tensor_scalar(
                    out=tle[:], in0=dbc[:, gs],
                    scalar1=iotaCh_t[:, kk : kk + 1], scalar2=None,
                    op0=AT.is_le)
                tge = gp.tile([128, 128], F16, name="tge", tag="tge", bufs=1)
                nc.vector.tensor_scalar(
                    out=tge[:], in0=debc[:, gs],
                    scalar1=iotaCh_t[:, kk : kk + 1], scalar2=None,
                    op0=AT.is_ge)
                band = gp.tile([128, 128], BF, name="band", tag="band", bufs=1)
                nc.vector.tensor_tensor(out=band[:], in0=tle[:], in1=tge[:],
                                        op=AT.mult)
                return band

            def prepass_group(g):
                KCg = kcs[g]
                dma(out=ea_all[:, g * KC : g * KC + KCg],
                    in_=AP(tensor=ea_h.tensor, offset=bases[g],
                           ap=[[1, 128], [128, KCg]]))
                sp = ps.tile([128, 128], F32, name="ssum", tag="rbc",
                             bufs=1)
                for kk in range(KCg):
                    band = build_band(g, kk)
                    nc.tensor.matmul(
                        sp[:, 0:1], band[:],
                        ea_all[:, g * KC + kk : g * KC + kk + 1],
                        start=(kk == 0), stop=(kk == KCg - 1))
                nc.vector.tensor_copy(out=ssum_sb[:, g : g + 1],
                                      in_=sp[:, 0:1])

            _done_g = [0]
            _aw1_stage = []
            nblocks = [(n0, min(512, T_cap - n0)) for n0 in range(0, T_cap, 512)]

            for n0, nw in nblocks:
                sTw = []
                eTw = []
                for k in range(8):
                    if n0 == 0:
                        # interleave weight/data DMAs so the first L1 chain's
                        # operands arrive earliest
                        t = pp.tile([128, HID], BF, name=f"wA{k}",
                                    tag=f"wA{k}")
                        dma(out=t[:], in_=aw1_p[k * 128 : (k + 1) * 128, :])
                        _aw1_stage.append(t)
                    ts_ = wst.tile([128, 512], BF, name=f"sTw{k}", tag=f"sTw{k}")
                    dma(out=ts_[:, :nw],
                        in_=statesT_p[k * 128 : (k + 1) * 128, n0 : n0 + nw])
                    sTw.append(ts_)
                aw1_t = _aw1_stage
                for k in range(8):
                    te_ = wst.tile([128, 512], BF, name=f"eTw{k}", tag=f"eTw{k}",
                                   bufs=1)
                    dma(out=te_[:, :nw],
                        in_=embedsT_p[k * 128 : (k + 1) * 128, n0 : n0 + nw])
                    eTw.append(te_)
                if aw2_t is None:
                    aw2_t = wload(aw2_p, "wB")
                h1a = [wst.tile([128, 512], BF, name=f"h1a{h}", tag=f"h1a{h}",
                                bufs=1)
                       for h in range(8)]
                h2a = [wst.tile([128, 512], BF, name=f"h2a{h}", tag=f"h2a{h}",
                                bufs=1)
                       for h in range(8)]
                for hc in range(8):
                    pt = ps.tile([128, 512], F32, name="big", tag="big", bufs=2)
                    for k in range(8):
                        nc.tensor.matmul(
                            pt[:, :nw],
                            aw1_t[k][:, hc * 128 : (hc + 1) * 128],
                            sTw[k][:, :nw], start=(k == 0), stop=(k == 7))
                    nc.scalar.activation(
                        h1a[hc][:, :nw], pt[:, :nw], AF.Relu,
                        bias=ab1_t[:, hc : hc + 1])
                for hc in range(8):
                    pt = ps.tile([128, 512], F32, name="big", tag="big", bufs=2)
                    for k in range(8):
                        nc.tensor.matmul(
                            pt[:, :nw],
                            aw2_t[k][:, hc * 128 : (hc + 1) * 128],
                            h1a[k][:, :nw], start=(k == 0), stop=(k == 7))
                    nc.scalar.activation(
                        h2a[hc][:, :nw], pt[:, :nw], AF.Relu,
                        bias=ab2_t[:, hc : hc + 1])
                pt = ps.tile([1, 512], F32, name="big", tag="big", bufs=2)
                for k in range(8):
                    nc.tensor.matmul(
                        pt[:, :nw], aw3_t[:, k : k + 1], h2a[k][:, :nw],
                        start=(k == 0), stop=(k == 7))
                nc.scalar.activation(ea_t[:, n0 : n0 + nw], pt[:, :nw], AF.Exp)
                dmas(out=ea_dram[n0 : n0 + nw], in_=ea_t[0:1, n0 : n0 + nw])
                # per-128-token projections into DRAM
                if w1a_t is None:
                    w1a_t = wload(w1a_p, "wWA")
                    w1b_t = wload(w1b_p, "wWB")
                    w1c_t = wload(w1c_p, "wWC")

                def proj(pi, wt_, srcs, j, scale_ap=None):
                    js = slice(j * 128, (j + 1) * 128)
                    for h0 in (0, 512):
                        pt = ps.tile([128, 512], F32, name="big",
                                     tag="big", bufs=2)
                        for k in range(8):
                            nc.tensor.matmul(
                                pt[:], srcs[k][:, js],
                                wt_[k][:, h0 : h0 + 512],
                                start=(k == 0), stop=(k == 7))
                        stg = wst.tile([128, 512], BF, name=f"pstg{pi}",
                                       tag=f"pstg{pi}", bufs=2)
                        if scale_ap is None:
                            nc.scalar.copy(stg[:], pt[:])
                        else:
                            nc.scalar.mul(stg[:], pt[:], scale_ap)
                        dmas(out=Pd[pi][n0 + j * 128 : n0 + (j + 1) * 128,
                                        h0 : h0 + 512], in_=stg[:])

                nj = nw // 128
                for j in range(nj):
                    proj(0, w1a_t, sTw, j)
                    proj(1, w1b_t, sTw, j)
                # exp(attn) columns for this block: [1,128] rows -> [128,1]
                eac_sb = wst.tile([128, 4], F32, name="eacs", tag="eacs",
                                  bufs=2)
                eac_ps = ps.tile([128, 128], F32, name="eac", tag="rbc",
                                 bufs=1)
                for j in range(nj):
                    nc.tensor.matmul(
                        eac_ps[:, j : j + 1],
                        ea_t[0:1, n0 + j * 128 : n0 + (j + 1) * 128],
                        ones11[:], start=True, stop=True)
                nc.vector.tensor_copy(out=eac_sb[:, :nj], in_=eac_ps[:, :nj])
                for j in range(nj):
                    proj(2, w1c_t, eTw, j, scale_ap=eac_sb[:, j : j + 1])
                while (_done_g[0] < G
                       and bases[_done_g[0]] + kcs[_done_g[0]] * 128
                       <= n0 + nw):
                    prepass_group(_done_g[0])
                    _done_g[0] += 1

            # pad P + ea beyond T_cap
            for pd in Pd:
                dma(out=pd[T_cap:, 0:512], in_=zrow[:])
                dma(out=pd[T_cap:, 512:1024], in_=zrow[:])
            zpad_t = pp.tile([1, 144], BF, name="zpad", tag="zpad")
            nc.vector.memset(zpad_t[:], 0.0)
            dma(out=ea_dram[T_cap:], in_=zpad_t[0:1, :])


            # ---------- WB = width_table @ W1d + b1  → [16, HID] bf16 ------
            WB_t = pp.tile([16, HID], BF, name="WB", tag="WB")
            for h0 in range(0, HID, 512):
                pt = ps.tile([16, 512], F32, name="big", tag="big", bufs=2)
                nc.tensor.matmul(pt[:], wtT_t[:], w1d_t[:, h0 : h0 + 512],
                                 start=True, stop=False)
                nc.tensor.matmul(pt[:], ones16_t[:], b1r_t[:, h0 : h0 + 512],
                                 start=False, stop=True)
                nc.vector.tensor_copy(out=WB_t[:, h0 : h0 + 512], in_=pt[:])

            # ---------- pre-pass epilogue: 1/ssum + transpose ----------
            rT_all = pp.tile([1, C], BF, name="rT_all", tag="rT_all")
            while _done_g[0] < G:
                prepass_group(_done_g[0])
                _done_g[0] += 1
            rinv32 = gp.tile([128, G], F32, name="rinv32", tag="rinv32")
            nc.vector.reciprocal(rinv32[:], ssum_sb[:])
            rinvbf = gp.tile([128, G], BF, name="rinvbf", tag="rinvbf")
            nc.vector.tensor_copy(out=rinvbf[:], in_=rinv32[:])
            rT8 = None
            for g in range(NGROUPS):
                if g % 8 == 0:
                    rT8 = ps.tile([128, 8, 128], BF, name="tr", tag="tr",
                                  bufs=1)
                nc.tensor.transpose(rT8[0:1, g % 8, :],
                                    rinvbf[:, g : g + 1], ident_t[:])
                if g % 8 == 7:
                    nc.vector.tensor_copy(
                        out=rT_all[:, (g - 7) * 128 : (g + 1) * 128],
                        in_=rT8[0:1, :, :])

            # ---------- span groups ----------
            w2_t = wload(w2_p, "wA")     # reuse token-weight slots for L2

            def prep_group(g):
                """One-hots + softmax weight band for group g (runs one
                group ahead of the consuming h1 chain)."""
                KCg = kcs[g]
                gs = slice(g * 128, (g + 1) * 128)
                rbc_ps = ps.tile([128, 128], F32, name="rbc", tag="rbc",
                                 bufs=1)
                nc.tensor.matmul(rbc_ps[:], ones128[:], rT_all[0:1, gs],
                                 start=True, stop=True)
                ohs, ohe, wt = [], [], []
                for kk in range(KCg):
                    o1 = gp.tile([128, 128], BF, name=f"ohs{kk}",
                                 tag=f"ohs{kk}", bufs=2)
                    nc.vector.tensor_scalar(
                        out=o1[:], in0=dbc[:, gs],
                        scalar1=iotaCh_t[:, kk : kk + 1], scalar2=None,
                        op0=AT.is_equal)
                    ohs.append(o1)
                    o2 = gp.tile([128, 128], BF, name=f"ohe{kk}",
                                 tag=f"ohe{kk}", bufs=2)
                    nc.vector.tensor_scalar(
                        out=o2[:], in0=debc[:, gs],
                        scalar1=iotaCh_t[:, kk : kk + 1], scalar2=None,
                        op0=AT.is_equal)
                    ohe.append(o2)
                    band = build_band(g, kk)
                    w_ = gp.tile([128, 128], BF, name=f"wt{kk}",
                                 tag=f"wt{kk}", bufs=2)
                    nc.vector.tensor_tensor(out=w_[:], in0=band[:],
                                            in1=rbc_ps[:], op=AT.mult)
                    wt.append(w_)
                ohl = gp.tile([16, 128], BF, name="ohl", tag="ohl", bufs=2)
                nc.vector.tensor_scalar(
                    out=ohl[:], in0=lnbc[:, gs],
                    scalar1=iotaCh_t[:16, 0:1], scalar2=None,
                    op0=AT.is_equal)
                return (ohs, ohe, wt, ohl)

            def pw_dmas(g):
                KCg = kcs[g]
                pw = []
                for pi in range(3):
                    tiles = []
                    for kk in range(KCg):
                        pt_ = wst.tile([128, HID], BF, name=f"pw{pi}_{kk}",
                                       tag=f"pw{pi}_{kk}", bufs=2)
                        r0 = bases[g] + kk * 128
                        dma(out=pt_[:], in_=Pd[pi][r0 : r0 + 128, :])
                        tiles.append(pt_)
                    pw.append(tiles)
                return pw

            h1b = h2b = None
            prep = prep_group(0)
            pwcur = pw_dmas(0)
            for g in range(NGROUPS):
                KCg = kcs[g]
                if g % 4 == 0:
                    h1b = gp.tile([128, 8, 512], BF, name="h1b", tag="h1b",
                                  bufs=1)
                gcol = (g % 4) * 128

                ohs, ohe, wt, ohl = prep
                pw = pwcur
                if g + 1 < NGROUPS:
                    pwcur = pw_dmas(g + 1)

                # h1[n, h]: one-hot stationary, P moving (dense M=512 MMs)
                hp = ps.tile([128, 1024], F32, name="h1f", tag="h1f",
                             bufs=2)
                steps = []
                for kk in range(KCg):
                    steps.append((ohs[kk][:], pw[0][kk]))
                    steps.append((ohe[kk][:], pw[1][kk]))
                steps.append((ohl[:], None))
                for kk in range(KCg):
                    steps.append((wt[kk][:], pw[2][kk]))
                ns = len(steps)
                for h0 in (0, 512):
                    for i, (lhsT, rhs_t) in enumerate(steps):
                        rhs = (WB_t[:, h0 : h0 + 512] if rhs_t is None
                               else rhs_t[:, h0 : h0 + 512])
                        nc.tensor.matmul(
                            hp[:, h0 : h0 + 512], lhsT, rhs,
                            start=(i == 0), stop=(i == ns - 1))
                h1s = gp.tile([128, 1024], BF, name="h1s", tag="h1s",
                              bufs=2)
                nc.vector.tensor_scalar(
                    out=h1s[:], in0=hp[:], scalar1=0.0, scalar2=None,
                    op0=AT.max)
                tr = ps.tile([128, 8, 128], BF, name="tr", tag="tr",
                             bufs=1)
                for k in range(8):
                    nc.tensor.transpose(
                        tr[:, k, :], h1s[:, k * 128 : (k + 1) * 128],
                        ident_t[:])
                nc.vector.tensor_copy(
                    out=h1b[:, :, gcol : gcol + 128], in_=tr[:])
                if g + 1 < NGROUPS:
                    prep = prep_group(g + 1)

                # every 4 groups: span-MLP L2+L3 on the 512-col block
                if g % 4 == 3:
                    b0 = (g // 4) * 512
                    h2b = [gp.tile([128, 512], BF, name=f"h2b{k}",
                                   tag=f"h2b{k}", bufs=1)
                           for k in range(8)]
                    for h2c in range(8):
                        pt = ps.tile([128, 512], F32, name="big", tag="big",
                                     bufs=2)
                        for k in range(8):
                            nc.tensor.matmul(
                                pt[:], w2_t[k][:, h2c * 128 : (h2c + 1) * 128],
                                h1b[:, k, :], start=(k == 0), stop=(k == 7))
                        nc.vector.tensor_scalar(
                            out=h2b[h2c][:], in0=pt[:],
                            scalar1=b2_t[:, h2c : h2c + 1], scalar2=0.0,
                            op0=AT.add, op1=AT.max)
                    pt = ps.tile([1, 512], F32, name="big", tag="big", bufs=2)
                    for k in range(8):
                        nc.tensor.matmul(pt[:], w3_t[:, k : k + 1], h2b[k][:],
                                         start=(k == 0), stop=(k == 7))
                    ob = gp.tile([1, 512], F32, name="ob", tag="ob")
                    nc.vector.tensor_scalar(out=ob[:], in0=pt[:],
                                            scalar1=float(b3val), scalar2=None,
                                            op0=AT.add)
                    dma(out=scores_p[:, b0 : b0 + 512], in_=ob[:])

    if SPLIT_WAITS:
        _split_waits(nc)
    return nc



def _split_waits(nc, max_waits=1):
    """This walrus build rejects instructions carrying >max_waits sem waits
    ("Too many sync wait commands"). Hoist excess waits onto same-engine
    NoOps placed immediately before the instruction — identical semantics
    (engine queues are in-order)."""
    ctr = [0]
    for f in nc.m.functions:
        for blk in f.blocks:
            out = []
            for ins in blk.instructions:
                si = getattr(ins, "sync_info", None)
                if si is not None and si.on_wait and len(si.on_wait) > max_waits:
                    waits = list(si.on_wait)
                    for w in waits[:-max_waits]:
                        ctr[0] += 1
                        nop = mybir.InstNoOp(
                            name=f"I-wsplit-{ctr[0]}", ins=[], outs=[],
                            sync_info=mybir.SyncInfo(on_wait=[w], on_update=[]),
                        )
                        nop.engine = ins.engine
                        out.append(nop)
                    ins.sync_info = mybir.SyncInfo(
                        on_wait=waits[-max_waits:],
                        on_update=list(si.on_update or []),
                    )
                out.append(ins)
            blk.instructions[:] = out
    return ctr[0]


_CACHE = {}
LAST_EXEC_NS = None
TRACE = False


def _install_ntff_shim():
    try:
        import antenv.axon_hooks  # noqa: F401
        return
    except ImportError:
        pass
    try:
        from trn_agent_boot.trn_boot import _ntff_profile_via_ctypes
        hook = _ntff_profile_via_ctypes("/opt/axon/libaxon_pjrt.so")
    except Exception:
        hook = None
    m1 = types.ModuleType("antenv")
    m2 = types.ModuleType("antenv.axon_hooks")
    m2.get_axon_ntff_profile_hook = lambda: hook
    m2.set_axon_ntff_profile_hook = lambda h: None
    m1.axon_hooks = m2
    sys.modules.setdefault("antenv", m1)
    sys.modules["antenv.axon_hooks"] = m2


def _prepare(inputs):
    inp = {k: np.asarray(v) for k, v in inputs.items()}
    ss = inp["span_starts"].astype(np.int64)
    sl = inp["span_lengths"].astype(np.int64)
    plan = _plan(ss, sl)
    T_cap, K_WIN, bases = plan["T_cap"], plan["K_WIN"], plan["bases"]
    KC = K_WIN // 128
    b3val = float(np.asarray(inp["score_b3"]).reshape(-1)[0])

    kcs = plan["kcs"]
    key = (T_cap, K_WIN, tuple(bases), tuple(kcs), b3val)
    if key not in _CACHE:
        _CACHE[key] = _build(T_cap, K_WIN, bases, kcs, b3val)
    nc = _CACHE[key]

    def bfc(x):
        return np.ascontiguousarray(np.asarray(x, dtype=np.float32)).astype(bf16)

    sw1 = inp["score_w1"].astype(np.float32)
    shared = {
        "aw1": bfc(inp["attn_w1"]),
        "aw2": bfc(inp["attn_w2"]),
        "aw3m": bfc(inp["attn_w3"].reshape(8, 128).T),
        "ab1m": np.ascontiguousarray(
            inp["attn_b1"].astype(np.float32).reshape(8, 128).T),
        "ab2m": np.ascontiguousarray(
            inp["attn_b2"].astype(np.float32).reshape(8, 128).T),
        "w1a": bfc(sw1[0:1024]),
        "w1b": bfc(sw1[1024:2048]),
        "w1c": bfc(sw1[2048:3072]),
        "w1d": bfc(sw1[3072:3092]),
        "wtT": bfc(inp["width_table"].T),
        "b1r": bfc(inp["score_b1"].reshape(1, HID)),
        "w2": bfc(inp["score_w2"]),
        "b2m": np.ascontiguousarray(
            inp["score_b2"].astype(np.float32).reshape(8, 128).T),
        "w3m": bfc(inp["score_w3"].reshape(8, 128).T),
        "iotaCh": np.ascontiguousarray(
            (np.arange(128, dtype=np.float32)[:, None]
             + 128.0 * np.arange(KC, dtype=np.float32)[None, :])
        ),
        "ident": np.eye(128, dtype=np.float32).astype(bf16),
    }

    states = inp["states"].astype(np.float32)
    embeds = inp["embeds"].astype(np.float32)
    in_maps = []
    for c in range(N_CORES):
        cb = int(plan["core_base"][c])
        stl = np.zeros((T_cap, D), np.float32)
        eml = np.zeros((T_cap, D), np.float32)
        hi = min(T, cb + T_cap)
        stl[: hi - cb] = states[cb:hi]
        eml[: hi - cb] = embeds[cb:hi]
        m = dict(shared)
        m["statesT"] = np.ascontiguousarray(stl.T).astype(bf16)
        m["embedsT"] = np.ascontiguousarray(eml.T).astype(bf16)
        d = plan["d"][c]
        dl = plan["dl"][c]
        ln = plan["ln"][c]
        m["dflat"] = d.reshape(1, C).astype(np.float16)
        m["deflat"] = dl.reshape(1, C).astype(np.float16)
        m["lenflat"] = ln.reshape(1, C).astype(np.float16)
        in_maps.append(m)

    return nc, in_maps, plan


def kernel(**inputs):
    global LAST_EXEC_NS
    from concourse.bass_utils import run_bass_kernel_spmd

    nc, in_maps, plan = _prepare(inputs)
    _install_ntff_shim()
    res = run_bass_kernel_spmd(nc, in_maps, list(range(N_CORES)), trace=TRACE)
    LAST_EXEC_NS = res.exec_time_ns

    out = np.empty(NSPAN, np.float32)
    for c in range(N_CORES):
        out[plan["order"][c * C : (c + 1) * C]] = np.asarray(
            res.results[c]["scores"]).reshape(-1)
    return out.reshape(NSPAN, 1)
